# revision 2
# baseline (speedup 1.0000x reference)
# GCN encoder (DGI) forward on 8 Trainium2 NeuronCores.
#
# Node-partitioned (graph-parallel) sharding:
#   - nodes are split contiguously across the 8 cores (N/8 per core)
#   - each core owns the edges whose *target* lands in its node range
#   - phase 1: every core computes xw' = dinv[s] * (x_s @ W_sn) for its own
#     nodes (bf16), then an AllGather replicates the full xw' table
#   - phase 2: each core gathers source rows for its edges with bulk indirect
#     DMA (bf16, 256B/row), scatter-adds them into per-window PSUM
#     accumulators with one-hot selector matmuls on the PE, and applies
#     dinv[t]/bias/PReLU in the epilogue.  Self-loops ride along as one
#     extra identity-selector chunk per window, gathered from the local
#     (pre-collective) table so they overlap the AllGather.
#
# Host-side work is limited to index preprocessing (edge routing/sorting,
# degree counting) and the tiny spectral-norm power iteration on W.

import ml_dtypes
import numpy as np

import concourse.bacc as bacc
import concourse.bass as bass
import concourse.mybir as mybir
import concourse.tile as tile
from concourse.bass_utils import run_bass_kernel_spmd

P = 128
F32 = mybir.dt.float32
BF16 = mybir.dt.bfloat16
I16 = mybir.dt.int16

# test-harness hooks (ignored in grading): set TRACE=True before calling
# kernel() to capture an NTFF profile; the BassKernelResults lands in
# LAST_RESULT.
TRACE = False
LAST_RESULT = None


def _l2n(v, eps=1e-12):
    return v / (np.linalg.norm(v) + eps)


def _spectral_norm_host(W, u):
    W = W.astype(np.float32)
    u = u.astype(np.float32)
    v = _l2n(W.T @ u)
    u2 = _l2n(W @ v)
    sigma = np.float32(u2 @ (W @ v))
    return W / sigma


def _prep_host(x, edge_index, n_cores, win_group, bucket_rows, max_call_chunks=0):
    """Route edges to cores by target and build the SPMD chunk schedule.

    Chunks are 128 edges, each mapping into one 128-target window and one
    source bucket (dma_gather has int16 indices, so the gathered table is
    addressed in buckets of `bucket_rows` rows).  Chunk order: for each
    super-group of `win_group` windows, first one self-loop chunk per
    window (bucket == -1, sourced from the local pre-collective table),
    then for each bucket the group's edge chunks.  One dma_gather call
    covers one (group, bucket) run.
    """
    n, nfeat = x.shape
    assert n % n_cores == 0
    npc = n // n_cores
    nwin = -(-npc // P)
    nbuck = -(-n // bucket_rows)
    assert bucket_rows < 32768

    row = np.ascontiguousarray(edge_index[0]).astype(np.int64)
    col = np.ascontiguousarray(edge_index[1]).astype(np.int64)

    # sort all edges by (target window, source bucket) so each (core, window,
    # bucket) run is contiguous; target order within a chunk is free (tloc).
    wkey = (col // npc) * nwin + (col % npc) // P  # global window id
    key = wkey * nbuck + row // bucket_rows
    order = np.argsort(key, kind="stable")
    rs = row[order]
    cs = col[order]
    cwb_sorted = key[order]

    deg = 1.0 + np.bincount(col, minlength=n).astype(np.float64)  # + self loop
    dinv_all = (deg ** -0.5).astype(np.float32)

    # counts per (core, window, bucket)
    cnt = np.bincount(key, minlength=n_cores * nwin * nbuck).reshape(
        n_cores, nwin, nbuck
    )
    kwb = -(-cnt // P)  # chunks per (c, w, b)
    kwb = kwb.max(axis=0)  # [nwin, nbuck] shared schedule

    # chunk order + gather-call runs; bucket -1 = self-loop (local table)
    chunk_win = []
    chunk_bucket = []
    call_sizes = []  # chunks per dma_gather call
    for wg in range(0, nwin, win_group):
        ws = list(range(wg, min(wg + win_group, nwin)))
        # self-loop chunks: one per window, first so they (a) open the
        # window's PSUM accumulation and (b) need no collective
        call_sizes.append(len(ws))
        for w in ws:
            chunk_win.append(w)
            chunk_bucket.append(-1)
        for b in range(nbuck):
            r = int(sum(kwb[w, b] for w in ws))
            if r == 0:
                continue
            if max_call_chunks > 0:
                q = r
                while q > 0:
                    call_sizes.append(min(q, max_call_chunks))
                    q -= max_call_chunks
            else:
                call_sizes.append(r)
            for w in ws:
                chunk_win.extend([w] * kwb[w, b])
                chunk_bucket.extend([b] * kwb[w, b])
    chunk_win = np.asarray(chunk_win)
    chunk_bucket = np.asarray(chunk_bucket)
    nchunks = len(chunk_win)

    # first/last chunk per window in this order
    first_of_win = np.zeros(nchunks, bool)
    last_of_win = np.zeros(nchunks, bool)
    seen = set()
    for j in range(nchunks):
        w = int(chunk_win[j])
        if w not in seen:
            first_of_win[j] = True
            seen.add(w)
    seen = set()
    for j in range(nchunks - 1, -1, -1):
        w = int(chunk_win[j])
        if w not in seen:
            last_of_win[j] = True
            seen.add(w)

    # first destination chunk per (w, b) (bucket runs only)
    base_by_wb = {}
    for j in range(nchunks):
        if chunk_bucket[j] < 0:
            continue
        key2 = (int(chunk_win[j]), int(chunk_bucket[j]))
        if key2 not in base_by_wb:
            base_by_wb[key2] = j

    # segment boundaries of (core, window, bucket) runs in the sorted list
    seg_lo_idx = np.searchsorted(
        cwb_sorted, np.arange(n_cores * nwin * nbuck), side="left"
    )
    seg_hi_idx = np.searchsorted(
        cwb_sorted, np.arange(n_cores * nwin * nbuck), side="right"
    )

    # self-loop chunk content (identical on every core; indices address the
    # local ag_in table of npc rows)
    sl_positions = np.where(chunk_bucket < 0)[0]

    src_cores = []
    tloc_cores = []
    dinv_cores = []
    for c in range(n_cores):
        src_flat = np.zeros(nchunks * P, np.int16)
        tloc_flat = np.full(nchunks * P, -1.0, np.float32)
        for j in sl_positions:
            w = int(chunk_win[j])
            nrow = min(P, npc - w * P)
            d0 = j * P
            src_flat[d0 : d0 + nrow] = (w * P + np.arange(nrow)).astype(np.int16)
            tloc_flat[d0 : d0 + nrow] = np.arange(nrow, dtype=np.float32)
        for w in range(nwin):
            for b in range(nbuck):
                if (w, b) not in base_by_wb:
                    continue
                s = c * nwin * nbuck + w * nbuck + b
                i0, i1 = seg_lo_idx[s], seg_hi_idx[s]
                m = i1 - i0
                if m == 0:
                    continue
                d0 = base_by_wb[(w, b)] * P
                src_flat[d0 : d0 + m] = (rs[i0:i1] - b * bucket_rows).astype(
                    np.int16
                )
                tloc_flat[d0 : d0 + m] = (cs[i0:i1] - c * npc - w * P).astype(
                    np.float32
                )
        # dma_gather idx layout: idx i -> partition i%16, col i//16,
        # replicated over the 8 groups of 16 partitions.
        a = src_flat.reshape(nchunks, 8, 16)  # [j, p//16, p%16]
        a = np.transpose(a, (2, 0, 1)).reshape(16, nchunks * 8)
        src_cores.append(np.ascontiguousarray(np.tile(a, (8, 1))))
        tloc_cores.append(np.ascontiguousarray(tloc_flat.reshape(nchunks, P).T))

        dv = np.zeros(nwin * P, np.float32)
        dv[:npc] = dinv_all[c * npc : (c + 1) * npc]
        dinv_cores.append(np.ascontiguousarray(dv.reshape(nwin, P).T))

    return dict(
        npc=npc,
        nwin=nwin,
        nbuck=nbuck,
        nchunks=nchunks,
        chunk_win=chunk_win,
        chunk_bucket=chunk_bucket,
        call_sizes=call_sizes,
        first_of_win=first_of_win,
        last_of_win=last_of_win,
        src_cores=src_cores,
        tloc_cores=tloc_cores,
        dinv_cores=dinv_cores,
    )


def _build_nc(
    n,
    nfeat,
    nhid,
    n_cores,
    nwin,
    nbuck,
    bucket_rows,
    nchunks,
    chunk_win,
    chunk_bucket,
    call_sizes,
    first_of_win,
    last_of_win,
    alpha,
    gather_bufs=6,
):
    npc_pad = nwin * P
    npc = n // n_cores
    assert nfeat % P == 0
    nk = nfeat // P  # contraction tiles for x @ W

    nc = bacc.Bacc(
        "TRN2",
        target_bir_lowering=False,
        debug=False,
        enable_asserts=False,
        num_devices=n_cores,
        num_swdge_queues=4,
    )

    x_in = nc.dram_tensor("x_sh", [npc_pad, nfeat], BF16, kind="ExternalInput")
    w_in = nc.dram_tensor("w_sn", [nfeat, nhid], BF16, kind="ExternalInput")
    dinv_in = nc.dram_tensor("dinv", [P, nwin], F32, kind="ExternalInput")
    bias_in = nc.dram_tensor("bias_t", [P, nhid], F32, kind="ExternalInput")
    max_call = max(call_sizes)
    iota_in = nc.dram_tensor("iota_t", [P, max_call * P], F32, kind="ExternalInput")
    src_in = nc.dram_tensor(
        "src_idx", [P, nchunks * 8], I16, kind="ExternalInput"
    )
    tloc_in = nc.dram_tensor("tloc", [P, nchunks], F32, kind="ExternalInput")
    out_d = nc.dram_tensor("out_sh", [npc_pad, nhid], F32, kind="ExternalOutput")

    assert sum(call_sizes) == nchunks

    with tile.TileContext(nc) as tc:
        with (
            tc.tile_pool(name="consts", bufs=1) as cpool,
            tc.tile_pool(name="dram", bufs=1, space="DRAM") as dpool,
        ):
            # constants
            w_sb = cpool.tile([P, nk, nhid], BF16)
            nc.sync.dma_start(
                w_sb[:], w_in[:].rearrange("(k p) h -> p k h", p=P)
            )
            bias_sb = cpool.tile([P, nhid], F32)
            nc.sync.dma_start(bias_sb[:], bias_in[:])
            iota_sb = cpool.tile([P, max_call * P], F32)
            nc.sync.dma_start(iota_sb[:], iota_in[:])
            dinv_sb = cpool.tile([P, nwin], F32)
            nc.sync.dma_start(dinv_sb[:], dinv_in[:])
            src_sb = cpool.tile([P, nchunks * 8], I16)
            nc.sync.dma_start(src_sb[:], src_in[:])
            tloc_sb = cpool.tile([P, nchunks], F32)
            nc.sync.dma_start(tloc_sb[:], tloc_in[:])

            ag_in = dpool.tile([npc, nhid], BF16)
            ag_out = dpool.tile([n, nhid], BF16, addr_space="Shared")

            # ---- phase 1: xw' = dinv[s] * (x_s @ W_sn) for owned nodes ----
            # x loads go through the DMA transpose xbar so the PE skips the
            # explicit transpose pass entirely (bf16-only feature).
            with (
                tc.tile_pool(name="p1xt", bufs=3) as xtpool,
                tc.tile_pool(name="p1o", bufs=3) as xwpool,
                tc.tile_pool(name="p1pm", bufs=2, space="PSUM") as psumXW,
            ):
                for w in range(nwin):
                    nrow = min(P, npc - w * P)
                    xT = xtpool.tile([P, nk, P], BF16)
                    for k in range(nk):
                        nc.sync.dma_start_transpose(
                            xT[:, k, :],
                            x_in[w * P : (w + 1) * P, k * P : (k + 1) * P],
                        )
                    pxw = psumXW.tile([P, nhid], F32)
                    for k in range(nk):
                        nc.tensor.matmul(
                            pxw[:],
                            lhsT=xT[:, k, :],
                            rhs=w_sb[:, k, :],
                            start=(k == 0),
                            stop=(k == nk - 1),
                        )
                    xwp = xwpool.tile([P, nhid], BF16)
                    nc.vector.tensor_scalar(
                        out=xwp[:],
                        in0=pxw[:],
                        scalar1=dinv_sb[:, w : w + 1],
                        scalar2=None,
                        op0=mybir.AluOpType.mult,
                    )
                    nc.sync.dma_start(
                        ag_in[w * P : w * P + nrow, :], xwp[:nrow, :]
                    )

            nc.gpsimd.collective_compute(
                "AllGather",
                mybir.AluOpType.bypass,
                replica_groups=[list(range(n_cores))],
                ins=[ag_in[:]],
                outs=[ag_out[:]],
            )

            # ---- phase 2: gather + one-hot matmul scatter-add + epilogue ----
            out_sb = cpool.tile([P, nwin * nhid], F32)
            psum_by_win = {}
            with (
                tc.tile_pool(name="gat", bufs=gather_bufs) as gpool,
                tc.tile_pool(name="sel", bufs=6) as spool,
                tc.tile_pool(name="tmp", bufs=4) as tpool,
                tc.tile_pool(name="acc", bufs=8, space="PSUM") as ppool,
            ):
                j = 0
                for ci, r in enumerate(call_sizes):
                    gbuf = gpool.tile(
                        [P, max_call * nhid], BF16, tag="gbuf", name="gbuf"
                    )
                    b = int(chunk_bucket[j])
                    if b < 0:
                        src_ap = ag_in[:, :]
                    else:
                        rows = min(bucket_rows, n - b * bucket_rows)
                        src_ap = ag_out[
                            b * bucket_rows : b * bucket_rows + rows, :
                        ]
                    nc.gpsimd.dma_gather(
                        gbuf[:, : r * nhid].rearrange("p (k e) -> p k e", e=nhid),
                        src_ap,
                        src_sb[:, j * 8 : (j + r) * 8],
                        r * P,
                        r * P,
                        nhid,
                        queue_num=ci % 4,
                    )
                    # one-hot selectors for the whole call in one DVE op
                    sel_big = spool.tile(
                        [P, max_call * P], BF16, tag="sel", name="sel_big"
                    )
                    nc.vector.tensor_tensor(
                        out=sel_big[:, : r * P].rearrange(
                            "p (k e) -> p k e", e=P
                        ),
                        in0=tloc_sb[:, j : j + r].to_broadcast([P, r, P]),
                        in1=iota_sb[:, : r * P].rearrange(
                            "p (k e) -> p k e", e=P
                        ),
                        op=mybir.AluOpType.is_equal,
                    )
                    for kk in range(r):
                        w = int(chunk_win[j])
                        if first_of_win[j]:
                            psum_by_win[w] = ppool.tile(
                                [P, nhid], F32, tag="pw", name="pw"
                            )
                        pw = psum_by_win[w]
                        nc.tensor.matmul(
                            pw[:],
                            lhsT=sel_big[:, kk * P : (kk + 1) * P],
                            rhs=gbuf[:, kk * nhid : (kk + 1) * nhid],
                            start=bool(first_of_win[j]),
                            stop=bool(last_of_win[j]),
                        )
                        if last_of_win[j]:
                            seg = out_sb[:, w * nhid : (w + 1) * nhid]
                            # dinv[t] * pw on ACT
                            nc.scalar.activation(
                                out=seg,
                                in_=pw[:],
                                func=mybir.ActivationFunctionType.Copy,
                                scale=dinv_sb[:, w : w + 1],
                            )
                            nc.vector.tensor_tensor(
                                out=seg,
                                in0=seg,
                                in1=bias_sb[:],
                                op=mybir.AluOpType.add,
                            )
                            # PReLU(y) = max(y, alpha*y) for 0 <= alpha <= 1
                            t2 = tpool.tile([P, nhid], F32, tag="t2", name="t2")
                            if 0.0 <= alpha <= 1.0:
                                nc.vector.tensor_scalar_mul(t2, seg, float(alpha))
                                nc.vector.tensor_tensor(
                                    out=seg,
                                    in0=seg,
                                    in1=t2,
                                    op=mybir.AluOpType.max,
                                )
                            else:
                                # general: max(y,0) + alpha*min(y,0)
                                nc.vector.tensor_scalar(
                                    out=t2,
                                    in0=seg,
                                    scalar1=0.0,
                                    scalar2=float(alpha),
                                    op0=mybir.AluOpType.min,
                                    op1=mybir.AluOpType.mult,
                                )
                                nc.vector.tensor_scalar_max(seg, seg, 0.0)
                                nc.vector.tensor_tensor(
                                    out=seg,
                                    in0=seg,
                                    in1=t2,
                                    op=mybir.AluOpType.add,
                                )
                        j += 1

            nc.sync.dma_start(
                out_d[:].rearrange("(w p) h -> p w h", p=P),
                out_sb[:].rearrange("p (w h) -> p w h", h=nhid),
            )

    nc.compile()
    return nc


def kernel(**inputs):
    x = np.asarray(inputs["x"], dtype=np.float32)
    edge_index = np.asarray(inputs["edge_index"])
    W = np.asarray(inputs["W"], dtype=np.float32)
    bias = np.asarray(inputs["bias"], dtype=np.float32)
    prelu_a = np.asarray(inputs["prelu_a"], dtype=np.float32)
    u = np.asarray(inputs["u"], dtype=np.float32)

    n, nfeat = x.shape
    nhid = W.shape[1]
    n_cores = 8
    win_group = 4
    nbuck = -(-n // 32767)  # int16 index reach per dma_gather bucket
    bucket_rows = -(-n // nbuck)
    alpha = float(prelu_a.reshape(-1)[0])

    # one dma_gather call must stay under the 1024-descriptor SWDGE ring
    # carveout (dynamic_dma_scratch_size//16); 7 chunks = 896 descriptors
    max_call_chunks = 7

    w_sn = _spectral_norm_host(W, u)
    prep = _prep_host(
        x, edge_index, n_cores, win_group, bucket_rows, max_call_chunks
    )
    npc, nwin, nchunks = prep["npc"], prep["nwin"], prep["nchunks"]

    nc = _build_nc(
        n,
        nfeat,
        nhid,
        n_cores,
        nwin,
        prep["nbuck"],
        bucket_rows,
        nchunks,
        prep["chunk_win"],
        prep["chunk_bucket"],
        prep["call_sizes"],
        prep["first_of_win"],
        prep["last_of_win"],
        alpha,
    )

    bias_t = np.ascontiguousarray(np.tile(bias[None, :], (P, 1)))
    max_call = max(prep["call_sizes"])
    iota_t = np.ascontiguousarray(
        np.tile(
            np.tile(np.arange(P, dtype=np.float32), max_call)[None, :], (P, 1)
        )
    )
    npc_pad = nwin * P
    w_bf = np.ascontiguousarray(w_sn.astype(ml_dtypes.bfloat16))

    in_maps = []
    for c in range(n_cores):
        x_sh = np.zeros((npc_pad, nfeat), ml_dtypes.bfloat16)
        x_sh[:npc] = x[c * npc : (c + 1) * npc].astype(ml_dtypes.bfloat16)
        in_maps.append(
            {
                "x_sh": x_sh,
                "w_sn": w_bf,
                "dinv": prep["dinv_cores"][c],
                "bias_t": bias_t,
                "iota_t": iota_t,
                "src_idx": prep["src_cores"][c],
                "tloc": prep["tloc_cores"][c],
            }
        )

    res = run_bass_kernel_spmd(
        nc, in_maps, core_ids=list(range(n_cores)), trace=TRACE
    )
    global LAST_RESULT
    LAST_RESULT = res
    out = np.concatenate(
        [res.results[c]["out_sh"][:npc] for c in range(n_cores)], axis=0
    )
    return out


# revision 7
# speedup vs baseline: 1.1860x; 1.1860x over previous
# GCN encoder (DGI) forward on 8 Trainium2 NeuronCores.
#
# Node-partitioned (graph-parallel) sharding:
#   - nodes are split contiguously across the 8 cores (N/8 per core)
#   - each core owns the edges whose *target* lands in its node range
#   - phase 1: every core computes xw' = dinv[s] * (x_s @ W_sn) for its own
#     nodes (bf16, x pre-transposed on host so no PE transposes), then an
#     AllGather replicates the full xw' table
#   - phase 2: each core gathers source rows for its edges with bulk indirect
#     DMA (bf16, 256B/row), scatter-adds them into per-window PSUM
#     accumulators with one-hot selector matmuls on the PE, and applies
#     dinv[t]*PReLU in a single ACT op per window.  Self-loops ride along as
#     one extra identity-selector chunk per window, gathered from the local
#     (pre-collective) table so they overlap the AllGather.
#
# Host-side work is limited to input layout (transpose/cast), index
# preprocessing (edge routing/sorting, degree counting) and the tiny
# spectral-norm power iteration on W.

import ml_dtypes
import numpy as np

import concourse.bacc as bacc
import concourse.bass as bass
import concourse.mybir as mybir
import concourse.tile as tile
from concourse.bass_utils import run_bass_kernel_spmd

P = 128
F32 = mybir.dt.float32
BF16 = mybir.dt.bfloat16
I16 = mybir.dt.int16

# test-harness hooks (ignored in grading)
TRACE = False
LAST_RESULT = None


def _l2n(v, eps=1e-12):
    return v / (np.linalg.norm(v) + eps)


def _spectral_norm_host(W, u):
    W = W.astype(np.float32)
    u = u.astype(np.float32)
    v = _l2n(W.T @ u)
    u2 = _l2n(W @ v)
    sigma = np.float32(u2 @ (W @ v))
    return W / sigma


def _prep_host(x, edge_index, n_cores, win_group, bucket_rows, max_call_chunks=0):
    """Route edges to cores by target and build the SPMD chunk schedule.

    Chunks are 128 edges, each mapping into one 128-target window and one
    source bucket (dma_gather has int16 indices, so the gathered table is
    addressed in buckets of `bucket_rows` rows).  Chunk order: for each
    super-group of `win_group` windows, first one self-loop chunk per
    window (bucket == -1, sourced from the local pre-collective table),
    then for each bucket the group's edge chunks.  One dma_gather call
    covers one (group, bucket) run.
    """
    n, nfeat = x.shape
    assert n % n_cores == 0
    npc = n // n_cores
    nwin = -(-npc // P)
    nbuck = -(-n // bucket_rows)
    assert bucket_rows < 32768

    row = np.ascontiguousarray(edge_index[0]).astype(np.int64)
    col = np.ascontiguousarray(edge_index[1]).astype(np.int64)

    wkey = (col // npc) * nwin + (col % npc) // P  # global window id
    key = wkey * nbuck + row // bucket_rows
    order = np.argsort(key, kind="stable")
    rs = row[order]
    cs = col[order]
    cwb_sorted = key[order]

    deg = 1.0 + np.bincount(col, minlength=n).astype(np.float64)  # + self loop
    dinv_all = (deg ** -0.5).astype(np.float32)

    cnt = np.bincount(key, minlength=n_cores * nwin * nbuck).reshape(
        n_cores, nwin, nbuck
    )
    kwb = -(-cnt // P)  # chunks per (c, w, b)
    kwb = kwb.max(axis=0)  # [nwin, nbuck] shared schedule

    chunk_win = []
    chunk_bucket = []
    call_sizes = []
    for wg in range(0, nwin, win_group):
        ws = list(range(wg, min(wg + win_group, nwin)))
        # self-loop chunks first: open each window's PSUM accumulation and
        # need no collective
        call_sizes.append(len(ws))
        for w in ws:
            chunk_win.append(w)
            chunk_bucket.append(-1)
        for b in range(nbuck):
            r = int(sum(kwb[w, b] for w in ws))
            if r == 0:
                continue
            if max_call_chunks > 0:
                q = r
                while q > 0:
                    call_sizes.append(min(q, max_call_chunks))
                    q -= max_call_chunks
            else:
                call_sizes.append(r)
            for w in ws:
                chunk_win.extend([w] * kwb[w, b])
                chunk_bucket.extend([b] * kwb[w, b])
    chunk_win = np.asarray(chunk_win)
    chunk_bucket = np.asarray(chunk_bucket)
    nchunks = len(chunk_win)

    first_of_win = np.zeros(nchunks, bool)
    last_of_win = np.zeros(nchunks, bool)
    seen = set()
    for j in range(nchunks):
        w = int(chunk_win[j])
        if w not in seen:
            first_of_win[j] = True
            seen.add(w)
    seen = set()
    for j in range(nchunks - 1, -1, -1):
        w = int(chunk_win[j])
        if w not in seen:
            last_of_win[j] = True
            seen.add(w)

    base_by_wb = {}
    for j in range(nchunks):
        if chunk_bucket[j] < 0:
            continue
        key2 = (int(chunk_win[j]), int(chunk_bucket[j]))
        if key2 not in base_by_wb:
            base_by_wb[key2] = j

    seg_lo_idx = np.searchsorted(
        cwb_sorted, np.arange(n_cores * nwin * nbuck), side="left"
    )
    seg_hi_idx = np.searchsorted(
        cwb_sorted, np.arange(n_cores * nwin * nbuck), side="right"
    )

    sl_positions = np.where(chunk_bucket < 0)[0]

    src_cores = []
    tloc_cores = []
    dinv_cores = []
    for c in range(n_cores):
        src_flat = np.zeros(nchunks * P, np.int16)
        tloc_flat = np.full(nchunks * P, -1.0, np.float32)
        for j in sl_positions:
            w = int(chunk_win[j])
            nrow = min(P, npc - w * P)
            d0 = j * P
            src_flat[d0 : d0 + nrow] = (w * P + np.arange(nrow)).astype(np.int16)
            tloc_flat[d0 : d0 + nrow] = np.arange(nrow, dtype=np.float32)
        for w in range(nwin):
            for b in range(nbuck):
                if (w, b) not in base_by_wb:
                    continue
                s = c * nwin * nbuck + w * nbuck + b
                i0, i1 = seg_lo_idx[s], seg_hi_idx[s]
                m = i1 - i0
                if m == 0:
                    continue
                d0 = base_by_wb[(w, b)] * P
                src_flat[d0 : d0 + m] = (rs[i0:i1] - b * bucket_rows).astype(
                    np.int16
                )
                tloc_flat[d0 : d0 + m] = (cs[i0:i1] - c * npc - w * P).astype(
                    np.float32
                )
        a = src_flat.reshape(nchunks, 8, 16)  # [j, p//16, p%16]
        a = np.transpose(a, (2, 0, 1)).reshape(16, nchunks * 8)
        src_cores.append(np.ascontiguousarray(np.tile(a, (8, 1))))
        tloc_cores.append(
            np.ascontiguousarray(
                tloc_flat.reshape(nchunks, P).T.astype(ml_dtypes.bfloat16)
            )
        )

        dv = np.zeros(nwin * P, np.float32)
        dv[:npc] = dinv_all[c * npc : (c + 1) * npc]
        dinv_cores.append(np.ascontiguousarray(dv.reshape(nwin, P).T))

    return dict(
        npc=npc,
        nwin=nwin,
        nbuck=nbuck,
        nchunks=nchunks,
        chunk_win=chunk_win,
        chunk_bucket=chunk_bucket,
        call_sizes=call_sizes,
        first_of_win=first_of_win,
        last_of_win=last_of_win,
        src_cores=src_cores,
        tloc_cores=tloc_cores,
        dinv_cores=dinv_cores,
    )


def _build_nc(
    n,
    nfeat,
    nhid,
    n_cores,
    nwin,
    nbuck,
    bucket_rows,
    nchunks,
    chunk_win,
    chunk_bucket,
    call_sizes,
    first_of_win,
    last_of_win,
    alpha,
    bias_is_zero,
    gather_bufs=6,
):
    npc_pad = nwin * P
    npc = n // n_cores
    assert nfeat % P == 0
    nk = nfeat // P

    nc = bacc.Bacc(
        "TRN2",
        target_bir_lowering=False,
        debug=False,
        enable_asserts=False,
        num_devices=n_cores,
        num_swdge_queues=4,
    )

    # x pre-transposed on host: [128, nk, npc_pad] bf16, xT[f, k, n] = x[n, 128k+f]
    xt_in = nc.dram_tensor("x_t", [P, nk, npc_pad], BF16, kind="ExternalInput")
    w_in = nc.dram_tensor("w_sn", [nfeat, nhid], BF16, kind="ExternalInput")
    dinv_in = nc.dram_tensor("dinv", [P, nwin], F32, kind="ExternalInput")
    bias_in = nc.dram_tensor("bias_t", [P, nhid], F32, kind="ExternalInput")
    max_call = max(call_sizes)
    iota_in = nc.dram_tensor("iota_t", [P, max_call * P], BF16, kind="ExternalInput")
    src_in = nc.dram_tensor("src_idx", [P, nchunks * 8], I16, kind="ExternalInput")
    tloc_in = nc.dram_tensor("tloc", [P, nchunks], BF16, kind="ExternalInput")
    out_d = nc.dram_tensor("out_sh", [npc_pad, nhid], F32, kind="ExternalOutput")

    assert sum(call_sizes) == nchunks

    with tile.TileContext(nc) as tc:
        with (
            tc.tile_pool(name="consts", bufs=1) as cpool,
            tc.tile_pool(name="dram", bufs=1, space="DRAM") as dpool,
        ):
            w_sb = cpool.tile([P, nk, nhid], BF16)
            nc.sync.dma_start(
                w_sb[:], w_in[:].rearrange("(k p) h -> p k h", p=P)
            )
            bias_sb = cpool.tile([P, nhid], F32)
            nc.sync.dma_start(bias_sb[:], bias_in[:])
            iota_sb = cpool.tile([P, max_call * P], BF16)
            nc.sync.dma_start(iota_sb[:], iota_in[:])
            dinv_sb = cpool.tile([P, nwin], F32)
            nc.sync.dma_start(dinv_sb[:], dinv_in[:])
            alpha_sb = cpool.tile([P, 1], F32)
            nc.vector.memset(alpha_sb[:], float(alpha))
            src_sb = cpool.tile([P, nchunks * 8], I16)
            nc.sync.dma_start(src_sb[:], src_in[:])
            tloc_sb = cpool.tile([P, nchunks], BF16)
            nc.sync.dma_start(tloc_sb[:], tloc_in[:])

            ag_in = dpool.tile([npc, nhid], BF16)
            ag_out = dpool.tile([n, nhid], BF16, addr_space="Shared")

            # ---- phase 1: xw' = dinv[s] * (x_s @ W_sn) for owned nodes ----
            with (
                tc.tile_pool(name="p1xt", bufs=4) as xtpool,
                tc.tile_pool(name="p1o", bufs=4) as xwpool,
                tc.tile_pool(name="p1pm", bufs=4, space="PSUM") as psumXW,
            ):
                for w in range(nwin):
                    nrow = min(P, npc - w * P)
                    xT = xtpool.tile([P, nk, P], BF16)
                    nc.sync.dma_start(
                        xT[:], xt_in[:, :, w * P : (w + 1) * P]
                    )
                    pxw = psumXW.tile([P, nhid], F32)
                    for k in range(nk):
                        nc.tensor.matmul(
                            pxw[:],
                            lhsT=xT[:, k, :],
                            rhs=w_sb[:, k, :],
                            start=(k == 0),
                            stop=(k == nk - 1),
                        )
                    xwp = xwpool.tile([P, nhid], BF16)
                    nc.scalar.activation(
                        out=xwp[:],
                        in_=pxw[:],
                        func=mybir.ActivationFunctionType.Copy,
                        scale=dinv_sb[:, w : w + 1],
                    )
                    nc.sync.dma_start(
                        ag_in[w * P : w * P + nrow, :], xwp[:nrow, :]
                    )

            nc.gpsimd.collective_compute(
                "AllGather",
                mybir.AluOpType.bypass,
                replica_groups=[list(range(n_cores))],
                ins=[ag_in[:]],
                outs=[ag_out[:]],
            )

            # ---- phase 2: gather + one-hot matmul scatter-add + epilogue ----
            out_sb = cpool.tile([P, nwin * nhid], F32)
            psum_by_win = {}
            with (
                tc.tile_pool(name="gat", bufs=gather_bufs) as gpool,
                tc.tile_pool(name="sel", bufs=6) as spool,
                tc.tile_pool(name="tmp", bufs=4) as tpool,
                tc.tile_pool(name="acc", bufs=8, space="PSUM") as ppool,
            ):
                j = 0
                for ci, r in enumerate(call_sizes):
                    gbuf = gpool.tile(
                        [P, max_call * nhid], BF16, tag="gbuf", name="gbuf"
                    )
                    b = int(chunk_bucket[j])
                    if b < 0:
                        src_ap = ag_in[:, :]
                    else:
                        rows = min(bucket_rows, n - b * bucket_rows)
                        src_ap = ag_out[
                            b * bucket_rows : b * bucket_rows + rows, :
                        ]
                    nc.gpsimd.dma_gather(
                        gbuf[:, : r * nhid].rearrange("p (k e) -> p k e", e=nhid),
                        src_ap,
                        src_sb[:, j * 8 : (j + r) * 8],
                        r * P,
                        r * P,
                        nhid,
                        queue_num=ci % 4,
                    )
                    sel_big = spool.tile(
                        [P, max_call * P], BF16, tag="sel", name="sel_big"
                    )
                    nc.vector.tensor_tensor(
                        out=sel_big[:, : r * P].rearrange(
                            "p (k e) -> p k e", e=P
                        ),
                        in0=tloc_sb[:, j : j + r].to_broadcast([P, r, P]),
                        in1=iota_sb[:, : r * P].rearrange(
                            "p (k e) -> p k e", e=P
                        ),
                        op=mybir.AluOpType.is_equal,
                    )
                    for kk in range(r):
                        w = int(chunk_win[j])
                        if first_of_win[j]:
                            psum_by_win[w] = ppool.tile(
                                [P, nhid], F32, tag="pw", name="pw"
                            )
                        pw = psum_by_win[w]
                        nc.tensor.matmul(
                            pw[:],
                            lhsT=sel_big[:, kk * P : (kk + 1) * P],
                            rhs=gbuf[:, kk * nhid : (kk + 1) * nhid],
                            start=bool(first_of_win[j]),
                            stop=bool(last_of_win[j]),
                        )
                        if last_of_win[j]:
                            seg = out_sb[:, w * nhid : (w + 1) * nhid]
                            if bias_is_zero and 0.0 <= alpha <= 1.0 and w % 2 == 0:
                                # out = PReLU(dinv_t * pw) in one ACT op
                                nc.scalar.activation(
                                    out=seg,
                                    in_=pw[:],
                                    func=mybir.ActivationFunctionType.Lrelu,
                                    scale=dinv_sb[:, w : w + 1],
                                    alpha=alpha_sb[:, 0:1],
                                )
                            else:
                                nc.scalar.activation(
                                    out=seg,
                                    in_=pw[:],
                                    func=mybir.ActivationFunctionType.Copy,
                                    scale=dinv_sb[:, w : w + 1],
                                )
                                nc.vector.tensor_tensor(
                                    out=seg,
                                    in0=seg,
                                    in1=bias_sb[:],
                                    op=mybir.AluOpType.add,
                                )
                                t2 = tpool.tile(
                                    [P, nhid], F32, tag="t2", name="t2"
                                )
                                if 0.0 <= alpha <= 1.0:
                                    nc.vector.tensor_scalar_mul(
                                        t2, seg, float(alpha)
                                    )
                                    nc.vector.tensor_tensor(
                                        out=seg,
                                        in0=seg,
                                        in1=t2,
                                        op=mybir.AluOpType.max,
                                    )
                                else:
                                    nc.vector.tensor_scalar(
                                        out=t2,
                                        in0=seg,
                                        scalar1=0.0,
                                        scalar2=float(alpha),
                                        op0=mybir.AluOpType.min,
                                        op1=mybir.AluOpType.mult,
                                    )
                                    nc.vector.tensor_scalar_max(seg, seg, 0.0)
                                    nc.vector.tensor_tensor(
                                        out=seg,
                                        in0=seg,
                                        in1=t2,
                                        op=mybir.AluOpType.add,
                                    )
                        j += 1

            nc.sync.dma_start(
                out_d[:].rearrange("(w p) h -> p w h", p=P),
                out_sb[:].rearrange("p (w h) -> p w h", h=nhid),
            )

    nc.compile()
    return nc


def kernel(**inputs):
    x = np.asarray(inputs["x"], dtype=np.float32)
    edge_index = np.asarray(inputs["edge_index"])
    W = np.asarray(inputs["W"], dtype=np.float32)
    bias = np.asarray(inputs["bias"], dtype=np.float32)
    prelu_a = np.asarray(inputs["prelu_a"], dtype=np.float32)
    u = np.asarray(inputs["u"], dtype=np.float32)

    n, nfeat = x.shape
    nhid = W.shape[1]
    n_cores = 8
    win_group = 6
    nbuck = -(-n // 32767)  # int16 index reach per dma_gather bucket
    bucket_rows = -(-n // nbuck)
    alpha = float(prelu_a.reshape(-1)[0])
    bias_is_zero = bool(np.all(bias == 0.0))

    # one dma_gather call must stay under the 1024-descriptor SWDGE ring
    max_call_chunks = 7

    w_sn = _spectral_norm_host(W, u)
    prep = _prep_host(
        x, edge_index, n_cores, win_group, bucket_rows, max_call_chunks
    )
    npc, nwin, nchunks = prep["npc"], prep["nwin"], prep["nchunks"]

    nc = _build_nc(
        n,
        nfeat,
        nhid,
        n_cores,
        nwin,
        prep["nbuck"],
        bucket_rows,
        nchunks,
        prep["chunk_win"],
        prep["chunk_bucket"],
        prep["call_sizes"],
        prep["first_of_win"],
        prep["last_of_win"],
        alpha,
        bias_is_zero,
    )

    bias_t = np.ascontiguousarray(np.tile(bias[None, :], (P, 1)))
    max_call = max(prep["call_sizes"])
    iota_t = np.ascontiguousarray(
        np.tile(
            np.tile(np.arange(P, dtype=np.float32), max_call)[None, :], (P, 1)
        ).astype(ml_dtypes.bfloat16)
    )
    npc_pad = nwin * P
    nk = nfeat // P
    w_bf = np.ascontiguousarray(w_sn.astype(ml_dtypes.bfloat16))

    in_maps = []
    for c in range(n_cores):
        xs = x[c * npc : (c + 1) * npc].astype(ml_dtypes.bfloat16)
        # xT[f, k, n] = x[n, 128k+f]
        xt = np.zeros((P, nk, npc_pad), ml_dtypes.bfloat16)
        xt[:, :, :npc] = np.transpose(
            xs.reshape(npc, nk, P), (2, 1, 0)
        )
        in_maps.append(
            {
                "x_t": np.ascontiguousarray(xt),
                "w_sn": w_bf,
                "dinv": prep["dinv_cores"][c],
                "bias_t": bias_t,
                "iota_t": iota_t,
                "src_idx": prep["src_cores"][c],
                "tloc": prep["tloc_cores"][c],
            }
        )

    res = run_bass_kernel_spmd(
        nc, in_maps, core_ids=list(range(n_cores)), trace=TRACE
    )
    global LAST_RESULT
    LAST_RESULT = res
    out = np.concatenate(
        [res.results[c]["out_sh"][:npc] for c in range(n_cores)], axis=0
    )
    return out


# revision 10
# speedup vs baseline: 1.5077x; 1.2712x over previous
# GCN encoder (DGI) forward on 8 Trainium2 NeuronCores.
#
# Node-partitioned (graph-parallel) sharding:
#   - nodes are split contiguously across the 8 cores (N/8 per core)
#   - each core owns the edges whose *target* lands in its node range
#   - phase 1: every core computes xw' = dinv[s] * (x_s @ W_sn) for its own
#     nodes (bf16, x pre-transposed on host), keeps it resident in SBUF and
#     AllGathers the full table to DRAM
#   - phase 2: per window, the self-loop lands first via an identity-selector
#     matmul from the resident local table (no DMA); edge messages are
#     gathered with bulk indirect DMA (bf16, 256B/row) from the AllGathered
#     table and scatter-added via one-hot selector matmuls into per-window
#     PSUM; epilogue applies dinv[t]/bias/PReLU.
#   - the gather schedule packs each (window-group, bucket) run contiguously
#     (no per-window chunk padding); chunks straddling window boundaries get
#     one selector matmul per (chunk, window) pair.
#
# Host-side work is limited to input layout (transpose/cast), index
# preprocessing (edge routing/sorting, degree counting) and the tiny
# spectral-norm power iteration on W.

import ml_dtypes
import numpy as np

import concourse.bacc as bacc
import concourse.bass as bass
import concourse.mybir as mybir
import concourse.tile as tile
from concourse.bass_utils import run_bass_kernel_spmd
from concourse.masks import make_identity

P = 128
F32 = mybir.dt.float32
BF16 = mybir.dt.bfloat16
I16 = mybir.dt.int16

# test-harness hooks (ignored in grading)
TRACE = False
LAST_RESULT = None


def _l2n(v, eps=1e-12):
    return v / (np.linalg.norm(v) + eps)


def _spectral_norm_host(W, u):
    W = W.astype(np.float32)
    u = u.astype(np.float32)
    v = _l2n(W.T @ u)
    u2 = _l2n(W @ v)
    sigma = np.float32(u2 @ (W @ v))
    return W / sigma


def _prep_host(x, edge_index, n_cores, win_group, bucket_rows, max_call_chunks=7):
    """Route edges to cores by target and build the SPMD chunk/pair schedule.

    Slot space: for each window-group g and source bucket b, the edges of
    the group's windows are laid out contiguously: window w owns slots
    [OFF[w], OFF[w]+M[w,b]) where M is the max edge count over cores
    (shared schedule).  Chunks are 128 consecutive slots; a dma_gather call
    covers up to `max_call_chunks` chunks of one (g, b) region.  A matmul
    "pair" is a (chunk, window) with a one-hot selector; chunks that
    straddle window boundaries carry one pair per window.
    """
    n, nfeat = x.shape
    assert n % n_cores == 0
    npc = n // n_cores
    nwin = -(-npc // P)
    nbuck = -(-n // bucket_rows)
    assert bucket_rows < 32768

    row = np.ascontiguousarray(edge_index[0]).astype(np.int64)
    col = np.ascontiguousarray(edge_index[1]).astype(np.int64)

    wkey = (col // npc) * nwin + (col % npc) // P  # global window id
    key = wkey * nbuck + row // bucket_rows
    order = np.argsort(key, kind="stable")
    rs = row[order]
    cs = col[order]
    cwb_sorted = key[order]

    deg = 1.0 + np.bincount(col, minlength=n).astype(np.float64)  # + self loop
    dinv_all = (deg ** -0.5).astype(np.float32)

    cnt = np.bincount(key, minlength=n_cores * nwin * nbuck).reshape(
        n_cores, nwin, nbuck
    )
    M = cnt.max(axis=0)  # [nwin, nbuck] shared slot counts

    seg_lo = np.searchsorted(
        cwb_sorted, np.arange(n_cores * nwin * nbuck), side="left"
    )

    # ---- shared schedule ----
    call_sizes = []  # chunks per call
    call_bucket = []
    pair_chunk = []  # global chunk id per pair
    pair_win = []
    pair_lo = []  # slot range of this pair inside its chunk [lo, hi)
    pair_hi = []
    chunk_off = []  # global slot offset (within its (g,b) region) per chunk
    chunk_region = []  # (g, b, region slot base handled via off arrays)
    # per-(g,b) bookkeeping for src fill
    regions = []  # (ws, b, OFF dict, L, chunk0)

    nchunks = 0
    for wg in range(0, nwin, win_group):
        ws = list(range(wg, min(wg + win_group, nwin)))
        for b in range(nbuck):
            OFF = {}
            L = 0
            for w in ws:
                OFF[w] = L
                L += int(M[w, b])
            if L == 0:
                continue
            nch = -(-L // 128)
            chunk0 = nchunks
            regions.append((ws, b, OFF, L, chunk0))
            q = nch
            while q > 0:
                r = min(q, max_call_chunks)
                call_sizes.append(r)
                call_bucket.append(b)
                q -= r
            for w in ws:
                m = int(M[w, b])
                if m == 0:
                    continue
                k0 = OFF[w] // 128
                k1 = (OFF[w] + m - 1) // 128
                for k in range(k0, k1 + 1):
                    lo = max(OFF[w], k * 128) - k * 128
                    hi = min(OFF[w] + m, (k + 1) * 128) - k * 128
                    pair_chunk.append(chunk0 + k)
                    pair_win.append(w)
                    pair_lo.append(lo)
                    pair_hi.append(hi)
            nchunks += nch

    pair_chunk = np.asarray(pair_chunk)
    pair_win = np.asarray(pair_win)
    npairs = len(pair_chunk)
    assert sum(call_sizes) == nchunks

    # last pair per window (stop + epilogue there); pairs are emitted in
    # schedule order, so a reverse scan suffices
    pair_last = np.zeros(npairs, bool)
    seen = set()
    for i in range(npairs - 1, -1, -1):
        w = int(pair_win[i])
        if w not in seen:
            pair_last[i] = True
            seen.add(w)

    # map call -> first chunk
    call_chunk0 = np.concatenate([[0], np.cumsum(call_sizes)[:-1]])

    # ---- per-core index/selector data ----
    src_cores = []
    tloc_cores = []
    dinv_cores = []
    for c in range(n_cores):
        src_flat = np.zeros(nchunks * P, np.int16)
        tloc_pairs = np.full((npairs, P), -1.0, np.float32)
        for ws, b, OFF, L, chunk0 in regions:
            base = chunk0 * P
            for w in ws:
                m_shared = int(M[w, b])
                if m_shared == 0:
                    continue
                s = c * nwin * nbuck + w * nbuck + b
                i0 = seg_lo[s]
                m = int(cnt[c, w, b])
                if m == 0:
                    continue
                d0 = base + OFF[w]
                src_flat[d0 : d0 + m] = (rs[i0 : i0 + m] - b * bucket_rows).astype(
                    np.int16
                )
        # fill tloc per pair
        for i in range(npairs):
            w = int(pair_win[i])
            k = int(pair_chunk[i])
            # region of this chunk
            # find region via chunk0: regions are in order
            # (precompute region per chunk instead)
            pass
        # vectorized tloc fill: iterate regions/windows once more
        pi = 0
        for ws, b, OFF, L, chunk0 in regions:
            for w in ws:
                m_shared = int(M[w, b])
                if m_shared == 0:
                    continue
                s = c * nwin * nbuck + w * nbuck + b
                i0 = seg_lo[s]
                m = int(cnt[c, w, b])
                k0 = OFF[w] // 128
                k1 = (OFF[w] + m_shared - 1) // 128
                for k in range(k0, k1 + 1):
                    lo = max(OFF[w], k * 128) - k * 128
                    hi = min(OFF[w] + m_shared, (k + 1) * 128) - k * 128
                    # slots [lo, hi) of chunk k belong to window w;
                    # core fills first m of the window's m_shared slots
                    gslot0 = k * 128 + lo  # region-relative slot of lo
                    e0 = gslot0 - OFF[w]  # edge offset within window run
                    ne = min(m - e0, hi - lo)
                    assert pair_chunk[pi] == chunk0 + k and pair_win[pi] == w
                    if ne > 0:
                        tloc_pairs[pi, lo : lo + ne] = (
                            cs[i0 + e0 : i0 + e0 + ne] - c * npc - w * P
                        ).astype(np.float32)
                    pi += 1
        assert pi == npairs

        a = src_flat.reshape(nchunks, 8, 16)
        a = np.transpose(a, (2, 0, 1)).reshape(16, nchunks * 8)
        src_cores.append(np.ascontiguousarray(np.tile(a, (8, 1))))
        tloc_cores.append(
            np.ascontiguousarray(tloc_pairs.T.astype(ml_dtypes.bfloat16))
        )

        dv = np.zeros(nwin * P, np.float32)
        dv[:npc] = dinv_all[c * npc : (c + 1) * npc]
        dinv_cores.append(np.ascontiguousarray(dv.reshape(nwin, P).T))

    return dict(
        npc=npc,
        nwin=nwin,
        nbuck=nbuck,
        nchunks=nchunks,
        npairs=npairs,
        call_sizes=call_sizes,
        call_bucket=call_bucket,
        call_chunk0=call_chunk0,
        pair_chunk=pair_chunk,
        pair_win=pair_win,
        pair_last=pair_last,
        win_group=win_group,
        src_cores=src_cores,
        tloc_cores=tloc_cores,
        dinv_cores=dinv_cores,
    )


def _build_nc(
    n,
    nfeat,
    nhid,
    n_cores,
    nwin,
    nbuck,
    bucket_rows,
    prep,
    alpha,
    bias_is_zero,
    gather_bufs=6,
):
    npc_pad = nwin * P
    npc = n // n_cores
    assert nfeat % P == 0
    nk = nfeat // P

    nchunks = prep["nchunks"]
    npairs = prep["npairs"]
    call_sizes = prep["call_sizes"]
    call_bucket = prep["call_bucket"]
    call_chunk0 = prep["call_chunk0"]
    pair_chunk = prep["pair_chunk"]
    pair_win = prep["pair_win"]
    pair_last = prep["pair_last"]
    win_group = prep["win_group"]

    nc = bacc.Bacc(
        "TRN2",
        target_bir_lowering=False,
        debug=False,
        enable_asserts=False,
        num_devices=n_cores,
        num_swdge_queues=4,
    )

    xt_in = nc.dram_tensor("x_t", [P, nk, npc_pad], BF16, kind="ExternalInput")
    w_in = nc.dram_tensor("w_sn", [nfeat, nhid], BF16, kind="ExternalInput")
    dinv_in = nc.dram_tensor("dinv", [P, nwin], F32, kind="ExternalInput")
    bias_in = nc.dram_tensor("bias_t", [P, nhid], F32, kind="ExternalInput")
    max_call = max(call_sizes)
    # pairs per call (for selector batch width)
    pairs_per_call = []
    for ci in range(len(call_sizes)):
        c0 = call_chunk0[ci]
        c1 = c0 + call_sizes[ci]
        pairs_per_call.append(
            int(np.sum((pair_chunk >= c0) & (pair_chunk < c1)))
        )
    max_pairs = max(pairs_per_call)
    iota_in = nc.dram_tensor(
        "iota_t", [P, max_pairs * P], BF16, kind="ExternalInput"
    )
    src_in = nc.dram_tensor("src_idx", [P, nchunks * 8], I16, kind="ExternalInput")
    tloc_in = nc.dram_tensor("tloc", [P, npairs], BF16, kind="ExternalInput")
    out_d = nc.dram_tensor("out_sh", [npc_pad, nhid], F32, kind="ExternalOutput")

    with tile.TileContext(nc) as tc:
        with (
            tc.tile_pool(name="consts", bufs=1) as cpool,
            tc.tile_pool(name="dram", bufs=1, space="DRAM") as dpool,
        ):
            w_sb = cpool.tile([P, nk, nhid], BF16)
            nc.sync.dma_start(
                w_sb[:], w_in[:].rearrange("(k p) h -> p k h", p=P)
            )
            bias_sb = cpool.tile([P, nhid], F32)
            nc.sync.dma_start(bias_sb[:], bias_in[:])
            iota_sb = cpool.tile([P, max_pairs * P], BF16)
            nc.sync.dma_start(iota_sb[:], iota_in[:])
            dinv_sb = cpool.tile([P, nwin], F32)
            nc.sync.dma_start(dinv_sb[:], dinv_in[:])
            src_sb = cpool.tile([P, nchunks * 8], I16)
            nc.sync.dma_start(src_sb[:], src_in[:])
            tloc_sb = cpool.tile([P, npairs], BF16)
            nc.sync.dma_start(tloc_sb[:], tloc_in[:])
            ident = cpool.tile([P, P], BF16)
            make_identity(nc, ident[:])

            # resident local table (written by phase 1, read by self-loop
            # matmuls) + DRAM staging for the collective
            xw_loc = cpool.tile([P, nwin * nhid], BF16)
            ag_in = dpool.tile([npc, nhid], BF16)
            ag_out = dpool.tile([n, nhid], BF16, addr_space="Shared")

            # ---- phase 1 ----
            with (
                tc.tile_pool(name="p1xt", bufs=4) as xtpool,
                tc.tile_pool(name="p1pm", bufs=4, space="PSUM") as psumXW,
            ):
                for w in range(nwin):
                    nrow = min(P, npc - w * P)
                    xT = xtpool.tile([P, nk, P], BF16)
                    nc.sync.dma_start(xT[:], xt_in[:, :, w * P : (w + 1) * P])
                    pxw = psumXW.tile([P, nhid], F32)
                    for k in range(nk):
                        nc.tensor.matmul(
                            pxw[:],
                            lhsT=xT[:, k, :],
                            rhs=w_sb[:, k, :],
                            start=(k == 0),
                            stop=(k == nk - 1),
                        )
                    seg = xw_loc[:, w * nhid : (w + 1) * nhid]
                    nc.scalar.activation(
                        out=seg,
                        in_=pxw[:],
                        func=mybir.ActivationFunctionType.Copy,
                        scale=dinv_sb[:, w : w + 1],
                    )
                    nc.sync.dma_start(
                        ag_in[w * P : w * P + nrow, :], seg[:nrow, :]
                    )

            nc.gpsimd.collective_compute(
                "AllGather",
                mybir.AluOpType.bypass,
                replica_groups=[list(range(n_cores))],
                ins=[ag_in[:]],
                outs=[ag_out[:]],
            )

            # ---- phase 2 ----
            out_sb = cpool.tile([P, nwin * nhid], F32)
            psum_by_win = {}
            started = set()
            with (
                tc.tile_pool(name="gat", bufs=gather_bufs) as gpool,
                tc.tile_pool(name="sel", bufs=6) as spool,
                tc.tile_pool(name="tmp", bufs=4) as tpool,
                tc.tile_pool(name="acc", bufs=8, space="PSUM") as ppool,
            ):
                pi = 0
                for ci, r in enumerate(call_sizes):
                    c0 = int(call_chunk0[ci])
                    b = int(call_bucket[ci])
                    gbuf = gpool.tile(
                        [P, max_call * nhid], BF16, tag="gbuf", name="gbuf"
                    )
                    rows = min(bucket_rows, n - b * bucket_rows)
                    nc.gpsimd.dma_gather(
                        gbuf[:, : r * nhid].rearrange("p (k e) -> p k e", e=nhid),
                        ag_out[b * bucket_rows : b * bucket_rows + rows, :],
                        src_sb[:, c0 * 8 : (c0 + r) * 8],
                        r * P,
                        r * P,
                        nhid,
                        queue_num=ci % 4,
                    )
                    np_call = pairs_per_call[ci]
                    sel_big = spool.tile(
                        [P, max_pairs * P], BF16, tag="sel", name="sel_big"
                    )
                    nc.vector.tensor_tensor(
                        out=sel_big[:, : np_call * P].rearrange(
                            "p (k e) -> p k e", e=P
                        ),
                        in0=tloc_sb[:, pi : pi + np_call].to_broadcast(
                            [P, np_call, P]
                        ),
                        in1=iota_sb[:, : np_call * P].rearrange(
                            "p (k e) -> p k e", e=P
                        ),
                        op=mybir.AluOpType.is_equal,
                    )
                    for q in range(np_call):
                        i = pi + q
                        w = int(pair_win[i])
                        kk = int(pair_chunk[i]) - c0
                        if w not in started:
                            started.add(w)
                            psum_by_win[w] = ppool.tile(
                                [P, nhid], F32, tag="pw", name="pw"
                            )
                            # self-loop via identity selector from the
                            # resident local table (no DMA involved)
                            nc.tensor.matmul(
                                psum_by_win[w][:],
                                lhsT=ident[:],
                                rhs=xw_loc[:, w * nhid : (w + 1) * nhid],
                                start=True,
                                stop=False,
                            )
                        pw = psum_by_win[w]
                        nc.tensor.matmul(
                            pw[:],
                            lhsT=sel_big[:, q * P : (q + 1) * P],
                            rhs=gbuf[:, kk * nhid : (kk + 1) * nhid],
                            start=False,
                            stop=bool(pair_last[i]),
                        )
                        if pair_last[i]:
                            seg = out_sb[:, w * nhid : (w + 1) * nhid]
                            nc.scalar.activation(
                                out=seg,
                                in_=pw[:],
                                func=mybir.ActivationFunctionType.Copy,
                                scale=dinv_sb[:, w : w + 1],
                            )
                            if not bias_is_zero:
                                nc.vector.tensor_tensor(
                                    out=seg,
                                    in0=seg,
                                    in1=bias_sb[:],
                                    op=mybir.AluOpType.add,
                                )
                            t2 = tpool.tile([P, nhid], F32, tag="t2", name="t2")
                            if 0.0 <= alpha <= 1.0:
                                nc.vector.tensor_scalar_mul(t2, seg, float(alpha))
                                nc.vector.tensor_tensor(
                                    out=seg,
                                    in0=seg,
                                    in1=t2,
                                    op=mybir.AluOpType.max,
                                )
                            else:
                                nc.vector.tensor_scalar(
                                    out=t2,
                                    in0=seg,
                                    scalar1=0.0,
                                    scalar2=float(alpha),
                                    op0=mybir.AluOpType.min,
                                    op1=mybir.AluOpType.mult,
                                )
                                nc.vector.tensor_scalar_max(seg, seg, 0.0)
                                nc.vector.tensor_tensor(
                                    out=seg,
                                    in0=seg,
                                    in1=t2,
                                    op=mybir.AluOpType.add,
                                )
                    pi += np_call
                assert pi == npairs

            nc.sync.dma_start(
                out_d[:].rearrange("(w p) h -> p w h", p=P),
                out_sb[:].rearrange("p (w h) -> p w h", h=nhid),
            )

    nc.compile()
    return nc


def kernel(**inputs):
    x = np.asarray(inputs["x"], dtype=np.float32)
    edge_index = np.asarray(inputs["edge_index"])
    W = np.asarray(inputs["W"], dtype=np.float32)
    bias = np.asarray(inputs["bias"], dtype=np.float32)
    prelu_a = np.asarray(inputs["prelu_a"], dtype=np.float32)
    u = np.asarray(inputs["u"], dtype=np.float32)

    n, nfeat = x.shape
    nhid = W.shape[1]
    n_cores = 8
    win_group = 6
    nbuck = -(-n // 32767)
    bucket_rows = -(-n // nbuck)
    alpha = float(prelu_a.reshape(-1)[0])
    bias_is_zero = bool(np.all(bias == 0.0))

    w_sn = _spectral_norm_host(W, u)
    prep = _prep_host(x, edge_index, n_cores, win_group, bucket_rows, 7)
    npc, nwin = prep["npc"], prep["nwin"]

    nc = _build_nc(
        n,
        nfeat,
        nhid,
        n_cores,
        nwin,
        prep["nbuck"],
        bucket_rows,
        prep,
        alpha,
        bias_is_zero,
    )

    bias_t = np.ascontiguousarray(np.tile(bias[None, :], (P, 1)))
    # iota sized to the max pairs per call
    max_pairs = 0
    for ci in range(len(prep["call_sizes"])):
        c0 = prep["call_chunk0"][ci]
        c1 = c0 + prep["call_sizes"][ci]
        max_pairs = max(
            max_pairs,
            int(
                np.sum(
                    (prep["pair_chunk"] >= c0) & (prep["pair_chunk"] < c1)
                )
            ),
        )
    iota_t = np.ascontiguousarray(
        np.tile(
            np.tile(np.arange(P, dtype=np.float32), max_pairs)[None, :], (P, 1)
        ).astype(ml_dtypes.bfloat16)
    )
    npc_pad = nwin * P
    nk = nfeat // P
    w_bf = np.ascontiguousarray(w_sn.astype(ml_dtypes.bfloat16))

    in_maps = []
    for c in range(n_cores):
        xs = x[c * npc : (c + 1) * npc].astype(ml_dtypes.bfloat16)
        xt = np.zeros((P, nk, npc_pad), ml_dtypes.bfloat16)
        xt[:, :, :npc] = np.transpose(xs.reshape(npc, nk, P), (2, 1, 0))
        in_maps.append(
            {
                "x_t": np.ascontiguousarray(xt),
                "w_sn": w_bf,
                "dinv": prep["dinv_cores"][c],
                "bias_t": bias_t,
                "iota_t": iota_t,
                "src_idx": prep["src_cores"][c],
                "tloc": prep["tloc_cores"][c],
            }
        )

    res = run_bass_kernel_spmd(
        nc, in_maps, core_ids=list(range(n_cores)), trace=TRACE
    )
    global LAST_RESULT
    LAST_RESULT = res
    out = np.concatenate(
        [res.results[c]["out_sh"][:npc] for c in range(n_cores)], axis=0
    )
    return out


# revision 14
# speedup vs baseline: 1.5746x; 1.0444x over previous
# GCN encoder (DGI) forward on 8 Trainium2 NeuronCores.
#
# Node-partitioned (graph-parallel) sharding:
#   - nodes are split contiguously across the 8 cores (N/8 per core)
#   - each core owns the edges whose *target* lands in its node range
#   - phase 1: every core computes xw' = dinv[s] * (x_s @ W_sn) for its own
#     nodes (bf16, x pre-transposed on host), keeps it resident in SBUF and
#     AllGathers the full table to DRAM
#   - phase 2: per window, the self-loop lands first via an identity-selector
#     matmul from the resident local table (no DMA); edge messages are
#     gathered with bulk indirect DMA (bf16, 256B/row) from the AllGathered
#     table and scatter-added via one-hot selector matmuls into per-window
#     PSUM; epilogue applies dinv[t]/bias/PReLU.
#   - the gather schedule packs each (window-group, bucket) run contiguously
#     (no per-window chunk padding); chunks straddling window boundaries get
#     one selector matmul per (chunk, window) pair.
#
# Host-side work is limited to input layout (transpose/cast), index
# preprocessing (edge routing/sorting, degree counting) and the tiny
# spectral-norm power iteration on W.

import ml_dtypes
import numpy as np

import concourse.bacc as bacc
import concourse.bass as bass
import concourse.mybir as mybir
import concourse.tile as tile
from concourse.bass_utils import run_bass_kernel_spmd
from concourse.masks import make_identity

P = 128
F32 = mybir.dt.float32
BF16 = mybir.dt.bfloat16
I16 = mybir.dt.int16

# test-harness hooks (ignored in grading)
TRACE = False
LAST_RESULT = None


def _l2n(v, eps=1e-12):
    return v / (np.linalg.norm(v) + eps)


def _spectral_norm_host(W, u):
    W = W.astype(np.float32)
    u = u.astype(np.float32)
    v = _l2n(W.T @ u)
    u2 = _l2n(W @ v)
    sigma = np.float32(u2 @ (W @ v))
    return W / sigma


def _prep_host(x, edge_index, n_cores, win_group, rows_a, max_call_chunks=7):
    """Route edges to cores by target and build the SPMD chunk/pair schedule.

    Slot space: for each window-group g and source bucket b, the edges of
    the group's windows are laid out contiguously: window w owns slots
    [OFF[w], OFF[w]+M[w,b]) where M is the max edge count over cores
    (shared schedule).  Chunks are 128 consecutive slots; a dma_gather call
    covers up to `max_call_chunks` chunks of one (g, b) region.  A matmul
    "pair" is a (chunk, window) with a one-hot selector; chunks that
    straddle window boundaries carry one pair per window.
    """
    n, nfeat = x.shape
    assert n % n_cores == 0
    npc = n // n_cores
    nwin = -(-npc // P)
    nbuck = -(-n // bucket_rows)
    assert bucket_rows < 32768

    row = np.ascontiguousarray(edge_index[0]).astype(np.int64)
    col = np.ascontiguousarray(edge_index[1]).astype(np.int64)

    wkey = (col // npc) * nwin + (col % npc) // P  # global window id
    key = wkey * nbuck + row // bucket_rows
    order = np.argsort(key, kind="stable")
    rs = row[order]
    cs = col[order]
    cwb_sorted = key[order]

    deg = 1.0 + np.bincount(col, minlength=n).astype(np.float64)  # + self loop
    dinv_all = (deg ** -0.5).astype(np.float32)

    cnt = np.bincount(key, minlength=n_cores * nwin * nbuck).reshape(
        n_cores, nwin, nbuck
    )
    M = cnt.max(axis=0)  # [nwin, nbuck] shared slot counts

    seg_lo = np.searchsorted(
        cwb_sorted, np.arange(n_cores * nwin * nbuck), side="left"
    )

    # ---- shared schedule ----
    call_sizes = []  # chunks per call
    call_bucket = []
    pair_chunk = []  # global chunk id per pair
    pair_win = []
    pair_lo = []  # slot range of this pair inside its chunk [lo, hi)
    pair_hi = []
    chunk_off = []  # global slot offset (within its (g,b) region) per chunk
    chunk_region = []  # (g, b, region slot base handled via off arrays)
    # per-(g,b) bookkeeping for src fill
    regions = []  # (ws, b, OFF dict, L, chunk0)

    nchunks = 0
    for wg in range(0, nwin, win_group):
        ws = list(range(wg, min(wg + win_group, nwin)))
        for b in range(nbuck):
            OFF = {}
            L = 0
            for w in ws:
                OFF[w] = L
                L += int(M[w, b])
            if L == 0:
                continue
            nch = -(-L // 128)
            chunk0 = nchunks
            regions.append((ws, b, OFF, L, chunk0))
            q = nch
            while q > 0:
                r = min(q, max_call_chunks)
                call_sizes.append(r)
                call_bucket.append(b)
                q -= r
            for w in ws:
                m = int(M[w, b])
                if m == 0:
                    continue
                k0 = OFF[w] // 128
                k1 = (OFF[w] + m - 1) // 128
                for k in range(k0, k1 + 1):
                    lo = max(OFF[w], k * 128) - k * 128
                    hi = min(OFF[w] + m, (k + 1) * 128) - k * 128
                    pair_chunk.append(chunk0 + k)
                    pair_win.append(w)
                    pair_lo.append(lo)
                    pair_hi.append(hi)
            nchunks += nch

    pair_chunk = np.asarray(pair_chunk)
    pair_win = np.asarray(pair_win)
    npairs = len(pair_chunk)
    assert sum(call_sizes) == nchunks

    # last pair per window (stop + epilogue there); pairs are emitted in
    # schedule order, so a reverse scan suffices
    pair_last = np.zeros(npairs, bool)
    seen = set()
    for i in range(npairs - 1, -1, -1):
        w = int(pair_win[i])
        if w not in seen:
            pair_last[i] = True
            seen.add(w)

    # map call -> first chunk
    call_chunk0 = np.concatenate([[0], np.cumsum(call_sizes)[:-1]])

    # ---- per-core index/selector data ----
    src_cores = []
    tloc_cores = []
    dinv_cores = []
    for c in range(n_cores):
        src_flat = np.zeros(nchunks * P, np.int16)
        tloc_pairs = np.full((npairs, P), -1.0, np.float32)
        for ws, b, OFF, L, chunk0 in regions:
            base = chunk0 * P
            for w in ws:
                m_shared = int(M[w, b])
                if m_shared == 0:
                    continue
                s = c * nwin * nbuck + w * nbuck + b
                i0 = seg_lo[s]
                m = int(cnt[c, w, b])
                if m == 0:
                    continue
                d0 = base + OFF[w]
                src_flat[d0 : d0 + m] = (rs[i0 : i0 + m] - b * bucket_rows).astype(
                    np.int16
                )
        # fill tloc per pair
        for i in range(npairs):
            w = int(pair_win[i])
            k = int(pair_chunk[i])
            # region of this chunk
            # find region via chunk0: regions are in order
            # (precompute region per chunk instead)
            pass
        # vectorized tloc fill: iterate regions/windows once more
        pi = 0
        for ws, b, OFF, L, chunk0 in regions:
            for w in ws:
                m_shared = int(M[w, b])
                if m_shared == 0:
                    continue
                s = c * nwin * nbuck + w * nbuck + b
                i0 = seg_lo[s]
                m = int(cnt[c, w, b])
                k0 = OFF[w] // 128
                k1 = (OFF[w] + m_shared - 1) // 128
                for k in range(k0, k1 + 1):
                    lo = max(OFF[w], k * 128) - k * 128
                    hi = min(OFF[w] + m_shared, (k + 1) * 128) - k * 128
                    # slots [lo, hi) of chunk k belong to window w;
                    # core fills first m of the window's m_shared slots
                    gslot0 = k * 128 + lo  # region-relative slot of lo
                    e0 = gslot0 - OFF[w]  # edge offset within window run
                    ne = min(m - e0, hi - lo)
                    assert pair_chunk[pi] == chunk0 + k and pair_win[pi] == w
                    if ne > 0:
                        tloc_pairs[pi, lo : lo + ne] = (
                            cs[i0 + e0 : i0 + e0 + ne] - c * npc - w * P
                        ).astype(np.float32)
                    pi += 1
        assert pi == npairs

        a = src_flat.reshape(nchunks, 8, 16)
        a = np.transpose(a, (2, 0, 1)).reshape(16, nchunks * 8)
        src_cores.append(np.ascontiguousarray(np.tile(a, (8, 1))))
        tloc_cores.append(
            np.ascontiguousarray(tloc_pairs.T.astype(ml_dtypes.bfloat16))
        )

        dv = np.zeros(nwin * P, np.float32)
        dv[:npc] = dinv_all[c * npc : (c + 1) * npc]
        dinv_cores.append(np.ascontiguousarray(dv.reshape(nwin, P).T))

    return dict(
        npc=npc,
        nwin=nwin,
        nbuck=nbuck,
        nchunks=nchunks,
        npairs=npairs,
        call_sizes=call_sizes,
        call_bucket=call_bucket,
        call_chunk0=call_chunk0,
        pair_chunk=pair_chunk,
        pair_win=pair_win,
        pair_last=pair_last,
        win_group=win_group,
        src_cores=src_cores,
        tloc_cores=tloc_cores,
        dinv_cores=dinv_cores,
    )


def _build_nc(
    n,
    nfeat,
    nhid,
    n_cores,
    nwin,
    nbuck,
    bucket_rows,
    prep,
    alpha,
    bias_is_zero,
    gather_bufs=8,
):
    npc_pad = nwin * P
    npc = n // n_cores
    assert nfeat % P == 0
    nk = nfeat // P

    nchunks = prep["nchunks"]
    npairs = prep["npairs"]
    call_sizes = prep["call_sizes"]
    call_bucket = prep["call_bucket"]
    call_chunk0 = prep["call_chunk0"]
    pair_chunk = prep["pair_chunk"]
    pair_win = prep["pair_win"]
    pair_last = prep["pair_last"]
    win_group = prep["win_group"]

    nc = bacc.Bacc(
        "TRN2",
        target_bir_lowering=False,
        debug=False,
        enable_asserts=False,
        num_devices=n_cores,
        num_swdge_queues=4,
    )

    xt_in = nc.dram_tensor("x_t", [P, nk, npc_pad], BF16, kind="ExternalInput")
    w_in = nc.dram_tensor("w_sn", [nfeat, nhid], BF16, kind="ExternalInput")
    dinv_in = nc.dram_tensor("dinv", [P, nwin], F32, kind="ExternalInput")
    bias_in = nc.dram_tensor("bias_t", [P, nhid], F32, kind="ExternalInput")
    max_call = max(call_sizes)
    # pairs per call (for selector batch width)
    pairs_per_call = []
    for ci in range(len(call_sizes)):
        c0 = call_chunk0[ci]
        c1 = c0 + call_sizes[ci]
        pairs_per_call.append(
            int(np.sum((pair_chunk >= c0) & (pair_chunk < c1)))
        )
    max_pairs = max(pairs_per_call)
    iota_in = nc.dram_tensor(
        "iota_t", [P, max_pairs * P], BF16, kind="ExternalInput"
    )
    src_in = nc.dram_tensor("src_idx", [P, nchunks * 8], I16, kind="ExternalInput")
    tloc_in = nc.dram_tensor("tloc", [P, npairs], BF16, kind="ExternalInput")
    out_d = nc.dram_tensor("out_sh", [npc_pad, nhid], F32, kind="ExternalOutput")

    with tile.TileContext(nc) as tc:
        with (
            tc.tile_pool(name="consts", bufs=1) as cpool,
            tc.tile_pool(name="dram", bufs=1, space="DRAM") as dpool,
        ):
            w_sb = cpool.tile([P, nk, nhid], BF16)
            nc.sync.dma_start(
                w_sb[:], w_in[:].rearrange("(k p) h -> p k h", p=P)
            )
            bias_sb = cpool.tile([P, nhid], F32)
            nc.sync.dma_start(bias_sb[:], bias_in[:])
            iota_sb = cpool.tile([P, max_pairs * P], BF16)
            nc.sync.dma_start(iota_sb[:], iota_in[:])
            dinv_sb = cpool.tile([P, nwin], F32)
            nc.sync.dma_start(dinv_sb[:], dinv_in[:])
            src_sb = cpool.tile([P, nchunks * 8], I16)
            nc.sync.dma_start(src_sb[:], src_in[:])
            tloc_sb = cpool.tile([P, npairs], BF16)
            nc.sync.dma_start(tloc_sb[:], tloc_in[:])
            ident = cpool.tile([P, P], BF16)
            make_identity(nc, ident[:])

            # resident local table (written by phase 1, read by self-loop
            # matmuls) + DRAM staging for the collective
            xw_loc = cpool.tile([P, nwin * nhid], BF16)
            ag_in = dpool.tile([npc, nhid], BF16)
            ag_out = dpool.tile([n, nhid], BF16, addr_space="Shared")

            # ---- phase 1 ----
            with (
                tc.tile_pool(name="p1xt", bufs=6) as xtpool,
                tc.tile_pool(name="p1pm", bufs=8, space="PSUM") as psumXW,
            ):
                for w in range(nwin):
                    nrow = min(P, npc - w * P)
                    xT = xtpool.tile([P, nk, P], BF16)
                    nc.sync.dma_start(xT[:], xt_in[:, :, w * P : (w + 1) * P])
                    pxw = psumXW.tile([P, nhid], F32)
                    for k in range(nk):
                        nc.tensor.matmul(
                            pxw[:],
                            lhsT=xT[:, k, :],
                            rhs=w_sb[:, k, :],
                            start=(k == 0),
                            stop=(k == nk - 1),
                        )
                    seg = xw_loc[:, w * nhid : (w + 1) * nhid]
                    nc.scalar.activation(
                        out=seg,
                        in_=pxw[:],
                        func=mybir.ActivationFunctionType.Copy,
                        scale=dinv_sb[:, w : w + 1],
                    )
                    nc.sync.dma_start(
                        ag_in[w * P : w * P + nrow, :], seg[:nrow, :]
                    )

            nc.gpsimd.collective_compute(
                "AllGather",
                mybir.AluOpType.bypass,
                replica_groups=[list(range(n_cores))],
                ins=[ag_in[:]],
                outs=[ag_out[:]],
            )

            # ---- phase 2 ----
            out_sb = cpool.tile([P, nwin * nhid], F32)
            psum_by_win = {}
            started = set()
            with (
                tc.tile_pool(name="gat", bufs=gather_bufs) as gpool,
                tc.tile_pool(name="sel", bufs=8) as spool,
                tc.tile_pool(name="tmp", bufs=4) as tpool,
                tc.tile_pool(name="acc", bufs=8, space="PSUM") as ppool,
            ):
                pi = 0
                for ci, r in enumerate(call_sizes):
                    c0 = int(call_chunk0[ci])
                    b = int(call_bucket[ci])
                    gbuf = gpool.tile(
                        [P, max_call * nhid], BF16, tag="gbuf", name="gbuf"
                    )
                    rows = min(bucket_rows, n - b * bucket_rows)
                    nc.gpsimd.dma_gather(
                        gbuf[:, : r * nhid].rearrange("p (k e) -> p k e", e=nhid),
                        ag_out[b * bucket_rows : b * bucket_rows + rows, :],
                        src_sb[:, c0 * 8 : (c0 + r) * 8],
                        r * P,
                        r * P,
                        nhid,
                        queue_num=ci % 4,
                    )
                    np_call = pairs_per_call[ci]
                    sel_big = spool.tile(
                        [P, max_pairs * P], BF16, tag="sel", name="sel_big"
                    )
                    nc.vector.tensor_tensor(
                        out=sel_big[:, : np_call * P].rearrange(
                            "p (k e) -> p k e", e=P
                        ),
                        in0=tloc_sb[:, pi : pi + np_call].to_broadcast(
                            [P, np_call, P]
                        ),
                        in1=iota_sb[:, : np_call * P].rearrange(
                            "p (k e) -> p k e", e=P
                        ),
                        op=mybir.AluOpType.is_equal,
                    )
                    for q in range(np_call):
                        i = pi + q
                        w = int(pair_win[i])
                        kk = int(pair_chunk[i]) - c0
                        if w not in started:
                            started.add(w)
                            psum_by_win[w] = ppool.tile(
                                [P, nhid], F32, tag="pw", name="pw"
                            )
                            # self-loop via identity selector from the
                            # resident local table (no DMA involved)
                            nc.tensor.matmul(
                                psum_by_win[w][:],
                                lhsT=ident[:],
                                rhs=xw_loc[:, w * nhid : (w + 1) * nhid],
                                start=True,
                                stop=False,
                            )
                        pw = psum_by_win[w]
                        nc.tensor.matmul(
                            pw[:],
                            lhsT=sel_big[:, q * P : (q + 1) * P],
                            rhs=gbuf[:, kk * nhid : (kk + 1) * nhid],
                            start=False,
                            stop=bool(pair_last[i]),
                        )
                        if pair_last[i]:
                            seg = out_sb[:, w * nhid : (w + 1) * nhid]
                            nc.scalar.activation(
                                out=seg,
                                in_=pw[:],
                                func=mybir.ActivationFunctionType.Copy,
                                scale=dinv_sb[:, w : w + 1],
                            )
                            if not bias_is_zero:
                                nc.vector.tensor_tensor(
                                    out=seg,
                                    in0=seg,
                                    in1=bias_sb[:],
                                    op=mybir.AluOpType.add,
                                )
                            t2 = tpool.tile([P, nhid], F32, tag="t2", name="t2")
                            if 0.0 <= alpha <= 1.0:
                                nc.vector.tensor_scalar_mul(t2, seg, float(alpha))
                                nc.vector.tensor_tensor(
                                    out=seg,
                                    in0=seg,
                                    in1=t2,
                                    op=mybir.AluOpType.max,
                                )
                            else:
                                nc.vector.tensor_scalar(
                                    out=t2,
                                    in0=seg,
                                    scalar1=0.0,
                                    scalar2=float(alpha),
                                    op0=mybir.AluOpType.min,
                                    op1=mybir.AluOpType.mult,
                                )
                                nc.vector.tensor_scalar_max(seg, seg, 0.0)
                                nc.vector.tensor_tensor(
                                    out=seg,
                                    in0=seg,
                                    in1=t2,
                                    op=mybir.AluOpType.add,
                                )
                    pi += np_call
                assert pi == npairs

            nc.sync.dma_start(
                out_d[:].rearrange("(w p) h -> p w h", p=P),
                out_sb[:].rearrange("p (w h) -> p w h", h=nhid),
            )

    nc.compile()
    return nc


def kernel(**inputs):
    x = np.asarray(inputs["x"], dtype=np.float32)
    edge_index = np.asarray(inputs["edge_index"])
    W = np.asarray(inputs["W"], dtype=np.float32)
    bias = np.asarray(inputs["bias"], dtype=np.float32)
    prelu_a = np.asarray(inputs["prelu_a"], dtype=np.float32)
    u = np.asarray(inputs["u"], dtype=np.float32)

    n, nfeat = x.shape
    nhid = W.shape[1]
    n_cores = 8
    win_group = 6
    nbuck = -(-n // 32767)
    bucket_rows = -(-n // nbuck)
    alpha = float(prelu_a.reshape(-1)[0])
    bias_is_zero = bool(np.all(bias == 0.0))

    w_sn = _spectral_norm_host(W, u)
    prep = _prep_host(x, edge_index, n_cores, win_group, bucket_rows, 7)
    npc, nwin = prep["npc"], prep["nwin"]

    nc = _build_nc(
        n,
        nfeat,
        nhid,
        n_cores,
        nwin,
        prep["nbuck"],
        bucket_rows,
        prep,
        alpha,
        bias_is_zero,
    )

    bias_t = np.ascontiguousarray(np.tile(bias[None, :], (P, 1)))
    # iota sized to the max pairs per call
    max_pairs = 0
    for ci in range(len(prep["call_sizes"])):
        c0 = prep["call_chunk0"][ci]
        c1 = c0 + prep["call_sizes"][ci]
        max_pairs = max(
            max_pairs,
            int(
                np.sum(
                    (prep["pair_chunk"] >= c0) & (prep["pair_chunk"] < c1)
                )
            ),
        )
    iota_t = np.ascontiguousarray(
        np.tile(
            np.tile(np.arange(P, dtype=np.float32), max_pairs)[None, :], (P, 1)
        ).astype(ml_dtypes.bfloat16)
    )
    npc_pad = nwin * P
    nk = nfeat // P
    w_bf = np.ascontiguousarray(w_sn.astype(ml_dtypes.bfloat16))

    in_maps = []
    for c in range(n_cores):
        xs = x[c * npc : (c + 1) * npc].astype(ml_dtypes.bfloat16)
        xt = np.zeros((P, nk, npc_pad), ml_dtypes.bfloat16)
        xt[:, :, :npc] = np.transpose(xs.reshape(npc, nk, P), (2, 1, 0))
        in_maps.append(
            {
                "x_t": np.ascontiguousarray(xt),
                "w_sn": w_bf,
                "dinv": prep["dinv_cores"][c],
                "bias_t": bias_t,
                "iota_t": iota_t,
                "src_idx": prep["src_cores"][c],
                "tloc": prep["tloc_cores"][c],
            }
        )

    res = run_bass_kernel_spmd(
        nc, in_maps, core_ids=list(range(n_cores)), trace=TRACE
    )
    global LAST_RESULT
    LAST_RESULT = res
    out = np.concatenate(
        [res.results[c]["out_sh"][:npc] for c in range(n_cores)], axis=0
    )
    return out


# revision 28
# speedup vs baseline: 1.6146x; 1.0254x over previous
# GCN encoder (DGI) forward on 8 Trainium2 NeuronCores.
#
# Node-partitioned (graph-parallel) sharding:
#   - nodes are split contiguously across the 8 cores (N/8 per core)
#   - each core owns the edges whose *target* lands in its node range
#   - phase 1: every core computes xw' = dinv[s] * (x_s @ W_sn) for its own
#     nodes (bf16, x pre-transposed on host), keeps it resident in SBUF and
#     AllGathers the full table to DRAM
#   - phase 2: per window, the self-loop lands first via an identity-selector
#     matmul from the resident local table (no DMA); edge messages are
#     gathered with bulk indirect DMA (bf16, 256B/row) from the AllGathered
#     table and scatter-added via one-hot selector matmuls into per-window
#     PSUM; epilogue applies dinv[t]/bias/PReLU.
#   - the gather schedule packs each (window-group, bucket) run contiguously
#     (no per-window chunk padding); chunks straddling window boundaries get
#     one selector matmul per (chunk, window) pair.
#
# Host-side work is limited to input layout (transpose/cast), index
# preprocessing (edge routing/sorting, degree counting) and the tiny
# spectral-norm power iteration on W.

import ml_dtypes
import numpy as np

import concourse.bacc as bacc
import concourse.bass as bass
import concourse.mybir as mybir
import concourse.tile as tile
from concourse.bass_utils import run_bass_kernel_spmd
from concourse.masks import make_identity

P = 128
F32 = mybir.dt.float32
BF16 = mybir.dt.bfloat16
I16 = mybir.dt.int16

# test-harness hooks (ignored in grading)
TRACE = False
LAST_RESULT = None


def _l2n(v, eps=1e-12):
    return v / (np.linalg.norm(v) + eps)


def _spectral_norm_host(W, u):
    W = W.astype(np.float32)
    u = u.astype(np.float32)
    v = _l2n(W.T @ u)
    u2 = _l2n(W @ v)
    sigma = np.float32(u2 @ (W @ v))
    return W / sigma


def _prep_host(x, edge_index, n_cores, win_group, rows_a, max_call_chunks=7):
    """Route edges to cores by target and build the SPMD chunk/pair schedule.

    Slot space: for each window-group g and source bucket b, the edges of
    the group's windows are laid out contiguously: window w owns slots
    [OFF[w], OFF[w]+M[w,b]) where M is the max edge count over cores
    (shared schedule).  Chunks are 128 consecutive slots; a dma_gather call
    covers up to `max_call_chunks` chunks of one (g, b) region.  A matmul
    "pair" is a (chunk, window) with a one-hot selector; chunks that
    straddle window boundaries carry one pair per window.
    """
    n, nfeat = x.shape
    assert n % n_cores == 0
    npc = n // n_cores
    nwin = -(-npc // P)
    # buckets: (table half, core quad) — half A is the first rows_a rows of
    # each core's shard (AllGathered first), half B the rest; four cores'
    # half-blocks are contiguous in the collective output, so one int16
    # gather bucket spans them (4*rows_a < 32768)
    nbuck = 4
    assert 4 * rows_a < 32768 and 4 * (npc - rows_a) < 32768

    row = np.ascontiguousarray(edge_index[0]).astype(np.int64)
    col = np.ascontiguousarray(edge_index[1]).astype(np.int64)

    wkey = (col // npc) * nwin + (col % npc) // P  # global window id
    s_loc = row % npc
    bkt = 2 * (s_loc >= rows_a) + (row // npc) // 4
    key = wkey * nbuck + bkt
    order = np.argsort(key, kind="stable")
    rs = row[order]
    cs = col[order]
    cwb_sorted = key[order]

    deg = 1.0 + np.bincount(col, minlength=n).astype(np.float64)  # + self loop
    dinv_all = (deg ** -0.5).astype(np.float32)

    cnt = np.bincount(key, minlength=n_cores * nwin * nbuck).reshape(
        n_cores, nwin, nbuck
    )
    M = cnt.max(axis=0)  # [nwin, nbuck] shared slot counts

    seg_lo = np.searchsorted(
        cwb_sorted, np.arange(n_cores * nwin * nbuck), side="left"
    )

    # ---- shared schedule ----
    call_sizes = []  # chunks per call
    call_bucket = []
    pair_chunk = []  # global chunk id per pair
    pair_win = []
    pair_lo = []  # slot range of this pair inside its chunk [lo, hi)
    pair_hi = []
    # per-(g,b) bookkeeping for src fill
    regions = []  # (ws, b, OFF dict, L, chunk0)

    # half-A buckets (0, 1) first: their gathers only need the first
    # collective and can start while half B is still exchanging
    border = [0, 1, 2, 3]
    nchunks = 0
    for wg in range(0, nwin, win_group):
        ws = list(range(wg, min(wg + win_group, nwin)))
        for b in border:
            OFF = {}
            L = 0
            for w in ws:
                OFF[w] = L
                L += int(M[w, b])
            if L == 0:
                continue
            nch = -(-L // 128)
            chunk0 = nchunks
            regions.append((ws, b, OFF, L, chunk0))
            q = nch
            while q > 0:
                r = min(q, max_call_chunks)
                call_sizes.append(r)
                call_bucket.append(b)
                q -= r
            for w in ws:
                m = int(M[w, b])
                if m == 0:
                    continue
                k0 = OFF[w] // 128
                k1 = (OFF[w] + m - 1) // 128
                for k in range(k0, k1 + 1):
                    lo = max(OFF[w], k * 128) - k * 128
                    hi = min(OFF[w] + m, (k + 1) * 128) - k * 128
                    pair_chunk.append(chunk0 + k)
                    pair_win.append(w)
                    pair_lo.append(lo)
                    pair_hi.append(hi)
            nchunks += nch

    pair_chunk = np.asarray(pair_chunk)
    pair_win = np.asarray(pair_win)
    npairs = len(pair_chunk)
    assert sum(call_sizes) == nchunks

    # last pair per window (stop + epilogue there); pairs are emitted in
    # schedule order, so a reverse scan suffices
    pair_last = np.zeros(npairs, bool)
    seen = set()
    for i in range(npairs - 1, -1, -1):
        w = int(pair_win[i])
        if w not in seen:
            pair_last[i] = True
            seen.add(w)

    # map call -> first chunk
    call_chunk0 = np.concatenate([[0], np.cumsum(call_sizes)[:-1]])

    # ---- per-core index/selector data ----
    src_cores = []
    tloc_cores = []
    dinv_cores = []
    for c in range(n_cores):
        src_flat = np.zeros(nchunks * P, np.int16)
        tloc_pairs = np.full((npairs, P), -1.0, np.float32)
        for ws, b, OFF, L, chunk0 in regions:
            base = chunk0 * P
            for w in ws:
                m_shared = int(M[w, b])
                if m_shared == 0:
                    continue
                s = c * nwin * nbuck + w * nbuck + b
                i0 = seg_lo[s]
                m = int(cnt[c, w, b])
                if m == 0:
                    continue
                d0 = base + OFF[w]
                h = b // 2
                rows_h = rows_a if h == 0 else npc - rows_a
                src_flat[d0 : d0 + m] = (
                    ((rs[i0 : i0 + m] // npc) % 4) * rows_h
                    + rs[i0 : i0 + m] % npc
                    - h * rows_a
                ).astype(np.int16)
        # fill tloc per pair: iterate regions/windows in schedule order
        pi = 0
        for ws, b, OFF, L, chunk0 in regions:
            for w in ws:
                m_shared = int(M[w, b])
                if m_shared == 0:
                    continue
                s = c * nwin * nbuck + w * nbuck + b
                i0 = seg_lo[s]
                m = int(cnt[c, w, b])
                k0 = OFF[w] // 128
                k1 = (OFF[w] + m_shared - 1) // 128
                for k in range(k0, k1 + 1):
                    lo = max(OFF[w], k * 128) - k * 128
                    hi = min(OFF[w] + m_shared, (k + 1) * 128) - k * 128
                    # slots [lo, hi) of chunk k belong to window w;
                    # core fills first m of the window's m_shared slots
                    gslot0 = k * 128 + lo  # region-relative slot of lo
                    e0 = gslot0 - OFF[w]  # edge offset within window run
                    ne = min(m - e0, hi - lo)
                    assert pair_chunk[pi] == chunk0 + k and pair_win[pi] == w
                    if ne > 0:
                        tloc_pairs[pi, lo : lo + ne] = (
                            cs[i0 + e0 : i0 + e0 + ne] - c * npc - w * P
                        ).astype(np.float32)
                    pi += 1
        assert pi == npairs

        a = src_flat.reshape(nchunks, 8, 16)
        a = np.transpose(a, (2, 0, 1)).reshape(16, nchunks * 8)
        src_cores.append(np.ascontiguousarray(np.tile(a, (8, 1))))
        tloc_cores.append(
            np.ascontiguousarray(tloc_pairs.T.astype(ml_dtypes.bfloat16))
        )

        dv = np.zeros(nwin * P, np.float32)
        dv[:npc] = dinv_all[c * npc : (c + 1) * npc]
        dinv_cores.append(np.ascontiguousarray(dv.reshape(nwin, P).T))

    return dict(
        npc=npc,
        nwin=nwin,
        nbuck=nbuck,
        nchunks=nchunks,
        npairs=npairs,
        call_sizes=call_sizes,
        call_bucket=call_bucket,
        call_chunk0=call_chunk0,
        pair_chunk=pair_chunk,
        pair_win=pair_win,
        pair_last=pair_last,
        win_group=win_group,
        src_cores=src_cores,
        tloc_cores=tloc_cores,
        dinv_cores=dinv_cores,
    )


def _build_nc(
    n,
    nfeat,
    nhid,
    n_cores,
    nwin,
    nbuck,
    rows_a,
    prep,
    alpha,
    bias_is_zero,
    gather_bufs=8,
):
    npc_pad = nwin * P
    npc = n // n_cores
    assert nfeat % P == 0
    nk = nfeat // P

    nchunks = prep["nchunks"]
    npairs = prep["npairs"]
    call_sizes = prep["call_sizes"]
    call_bucket = prep["call_bucket"]
    call_chunk0 = prep["call_chunk0"]
    pair_chunk = prep["pair_chunk"]
    pair_win = prep["pair_win"]
    pair_last = prep["pair_last"]
    win_group = prep["win_group"]

    nc = bacc.Bacc(
        "TRN2",
        target_bir_lowering=False,
        debug=False,
        enable_asserts=False,
        num_devices=n_cores,
        num_swdge_queues=4,
    )

    xt_in = nc.dram_tensor("x_t", [P, nk, npc_pad], BF16, kind="ExternalInput")
    w_in = nc.dram_tensor("w_sn", [nfeat, nhid], BF16, kind="ExternalInput")
    dinv_in = nc.dram_tensor("dinv", [P, nwin], F32, kind="ExternalInput")
    bias_in = nc.dram_tensor("bias_t", [P, nhid], F32, kind="ExternalInput")
    max_call = max(call_sizes)
    # pairs per call (for selector batch width)
    pairs_per_call = []
    for ci in range(len(call_sizes)):
        c0 = call_chunk0[ci]
        c1 = c0 + call_sizes[ci]
        pairs_per_call.append(
            int(np.sum((pair_chunk >= c0) & (pair_chunk < c1)))
        )
    max_pairs = max(pairs_per_call)
    iota_in = nc.dram_tensor(
        "iota_t", [P, max_pairs * P], BF16, kind="ExternalInput"
    )
    src_in = nc.dram_tensor("src_idx", [P, nchunks * 8], I16, kind="ExternalInput")
    tloc_in = nc.dram_tensor("tloc", [P, npairs], BF16, kind="ExternalInput")
    out_d = nc.dram_tensor("out_sh", [npc_pad, nhid], F32, kind="ExternalOutput")

    with tile.TileContext(nc) as tc:
        with (
            tc.tile_pool(name="consts", bufs=1) as cpool,
            tc.tile_pool(name="dram", bufs=1, space="DRAM") as dpool,
        ):
            w_sb = cpool.tile([P, nk, nhid], BF16)
            nc.sync.dma_start(
                w_sb[:], w_in[:].rearrange("(k p) h -> p k h", p=P)
            )
            bias_sb = cpool.tile([P, nhid], F32)
            nc.sync.dma_start(bias_sb[:], bias_in[:])
            iota_sb = cpool.tile([P, max_pairs * P], BF16)
            nc.sync.dma_start(iota_sb[:], iota_in[:])
            dinv_sb = cpool.tile([P, nwin], F32)
            nc.sync.dma_start(dinv_sb[:], dinv_in[:])
            src_sb = cpool.tile([P, nchunks * 8], I16)
            nc.sync.dma_start(src_sb[:], src_in[:])
            tloc_sb = cpool.tile([P, npairs], BF16)
            nc.sync.dma_start(tloc_sb[:], tloc_in[:])
            ident = cpool.tile([P, P], BF16)
            make_identity(nc, ident[:])

            # resident local table (written by phase 1, read by self-loop
            # matmuls) + DRAM staging for the two half-table collectives
            rows_b = npc - rows_a
            wins_a = rows_a // P
            xw_loc = cpool.tile([P, nwin * nhid], BF16)
            ag_in_a = dpool.tile([rows_a, nhid], BF16)
            ag_in_b = dpool.tile([rows_b, nhid], BF16)
            ag_out_a = dpool.tile([n_cores * rows_a, nhid], BF16, addr_space="Shared")
            ag_out_b = dpool.tile([n_cores * rows_b, nhid], BF16, addr_space="Shared")

            # ---- phase 1 ----
            with (
                tc.tile_pool(name="p1xt", bufs=6) as xtpool,
                tc.tile_pool(name="p1pm", bufs=8, space="PSUM") as psumXW,
            ):
                for w in range(nwin):
                    nrow = min(P, npc - w * P)
                    xT = xtpool.tile([P, nk, P], BF16)
                    nc.sync.dma_start(xT[:], xt_in[:, :, w * P : (w + 1) * P])
                    pxw = psumXW.tile([P, nhid], F32)
                    for k in range(nk):
                        nc.tensor.matmul(
                            pxw[:],
                            lhsT=xT[:, k, :],
                            rhs=w_sb[:, k, :],
                            start=(k == 0),
                            stop=(k == nk - 1),
                        )
                    seg = xw_loc[:, w * nhid : (w + 1) * nhid]
                    nc.scalar.activation(
                        out=seg,
                        in_=pxw[:],
                        func=mybir.ActivationFunctionType.Copy,
                        scale=dinv_sb[:, w : w + 1],
                    )
                    if w < wins_a:
                        nc.sync.dma_start(
                            ag_in_a[w * P : w * P + nrow, :], seg[:nrow, :]
                        )
                    else:
                        r0 = (w - wins_a) * P
                        nc.sync.dma_start(
                            ag_in_b[r0 : r0 + nrow, :], seg[:nrow, :]
                        )

            nc.gpsimd.collective_compute(
                "AllGather",
                mybir.AluOpType.bypass,
                replica_groups=[list(range(n_cores))],
                ins=[ag_in_a[:]],
                outs=[ag_out_a[:]],
            )
            nc.gpsimd.collective_compute(
                "AllGather",
                mybir.AluOpType.bypass,
                replica_groups=[list(range(n_cores))],
                ins=[ag_in_b[:]],
                outs=[ag_out_b[:]],
            )

            # ---- phase 2 ----
            out_sb = cpool.tile([P, nwin * nhid], F32)
            psum_by_win = {}
            started = set()
            with (
                tc.tile_pool(name="gat", bufs=gather_bufs) as gpool,
                tc.tile_pool(name="sel", bufs=8) as spool,
                tc.tile_pool(name="tmp", bufs=4) as tpool,
                tc.tile_pool(name="acc", bufs=8, space="PSUM") as ppool,
            ):
                pi = 0
                for ci, r in enumerate(call_sizes):
                    c0 = int(call_chunk0[ci])
                    b = int(call_bucket[ci])
                    gbuf = gpool.tile(
                        [P, max_call * nhid], BF16, tag="gbuf", name="gbuf"
                    )
                    h, q = b >> 1, b & 1
                    if h == 0:
                        src_ap = ag_out_a[
                            q * 4 * rows_a : (q + 1) * 4 * rows_a, :
                        ]
                    else:
                        src_ap = ag_out_b[
                            q * 4 * rows_b : (q + 1) * 4 * rows_b, :
                        ]
                    nc.gpsimd.dma_gather(
                        gbuf[:, : r * nhid].rearrange("p (k e) -> p k e", e=nhid),
                        src_ap,
                        src_sb[:, c0 * 8 : (c0 + r) * 8],
                        r * P,
                        r * P,
                        nhid,
                        queue_num=ci % 4,
                    )
                    np_call = pairs_per_call[ci]
                    sel_big = spool.tile(
                        [P, max_pairs * P], BF16, tag="sel", name="sel_big"
                    )
                    nc.vector.tensor_tensor(
                        out=sel_big[:, : np_call * P].rearrange(
                            "p (k e) -> p k e", e=P
                        ),
                        in0=tloc_sb[:, pi : pi + np_call].to_broadcast(
                            [P, np_call, P]
                        ),
                        in1=iota_sb[:, : np_call * P].rearrange(
                            "p (k e) -> p k e", e=P
                        ),
                        op=mybir.AluOpType.is_equal,
                    )
                    for q in range(np_call):
                        i = pi + q
                        w = int(pair_win[i])
                        kk = int(pair_chunk[i]) - c0
                        if w not in started:
                            started.add(w)
                            psum_by_win[w] = ppool.tile(
                                [P, nhid], F32, tag="pw", name="pw"
                            )
                            # self-loop via identity selector from the
                            # resident local table (no DMA involved)
                            nc.tensor.matmul(
                                psum_by_win[w][:],
                                lhsT=ident[:],
                                rhs=xw_loc[:, w * nhid : (w + 1) * nhid],
                                start=True,
                                stop=False,
                            )
                        pw = psum_by_win[w]
                        nc.tensor.matmul(
                            pw[:],
                            lhsT=sel_big[:, q * P : (q + 1) * P],
                            rhs=gbuf[:, kk * nhid : (kk + 1) * nhid],
                            start=False,
                            stop=bool(pair_last[i]),
                        )
                        if pair_last[i]:
                            seg = out_sb[:, w * nhid : (w + 1) * nhid]
                            nc.scalar.activation(
                                out=seg,
                                in_=pw[:],
                                func=mybir.ActivationFunctionType.Copy,
                                scale=dinv_sb[:, w : w + 1],
                            )
                            if not bias_is_zero:
                                nc.vector.tensor_tensor(
                                    out=seg,
                                    in0=seg,
                                    in1=bias_sb[:],
                                    op=mybir.AluOpType.add,
                                )
                            t2 = tpool.tile([P, nhid], F32, tag="t2", name="t2")
                            if 0.0 <= alpha <= 1.0:
                                nc.vector.tensor_scalar_mul(t2, seg, float(alpha))
                                nc.vector.tensor_tensor(
                                    out=seg,
                                    in0=seg,
                                    in1=t2,
                                    op=mybir.AluOpType.max,
                                )
                            else:
                                nc.vector.tensor_scalar(
                                    out=t2,
                                    in0=seg,
                                    scalar1=0.0,
                                    scalar2=float(alpha),
                                    op0=mybir.AluOpType.min,
                                    op1=mybir.AluOpType.mult,
                                )
                                nc.vector.tensor_scalar_max(seg, seg, 0.0)
                                nc.vector.tensor_tensor(
                                    out=seg,
                                    in0=seg,
                                    in1=t2,
                                    op=mybir.AluOpType.add,
                                )
                    pi += np_call
                assert pi == npairs

            nc.sync.dma_start(
                out_d[:].rearrange("(w p) h -> p w h", p=P),
                out_sb[:].rearrange("p (w h) -> p w h", h=nhid),
            )

    nc.compile()
    return nc


def kernel(**inputs):
    x = np.asarray(inputs["x"], dtype=np.float32)
    edge_index = np.asarray(inputs["edge_index"])
    W = np.asarray(inputs["W"], dtype=np.float32)
    bias = np.asarray(inputs["bias"], dtype=np.float32)
    prelu_a = np.asarray(inputs["prelu_a"], dtype=np.float32)
    u = np.asarray(inputs["u"], dtype=np.float32)

    n, nfeat = x.shape
    nhid = W.shape[1]
    n_cores = 8
    win_group = 6
    npc0 = n // n_cores
    # half-A of each core's shard = 49 windows (window-aligned split)
    rows_a = min((-(-npc0 // P) // 2 + 1) * P, npc0)
    alpha = float(prelu_a.reshape(-1)[0])
    bias_is_zero = bool(np.all(bias == 0.0))

    w_sn = _spectral_norm_host(W, u)
    prep = _prep_host(x, edge_index, n_cores, win_group, rows_a, 7)
    npc, nwin = prep["npc"], prep["nwin"]

    nc = _build_nc(
        n,
        nfeat,
        nhid,
        n_cores,
        nwin,
        prep["nbuck"],
        rows_a,
        prep,
        alpha,
        bias_is_zero,
    )

    bias_t = np.ascontiguousarray(np.tile(bias[None, :], (P, 1)))
    # iota sized to the max pairs per call
    max_pairs = 0
    for ci in range(len(prep["call_sizes"])):
        c0 = prep["call_chunk0"][ci]
        c1 = c0 + prep["call_sizes"][ci]
        max_pairs = max(
            max_pairs,
            int(
                np.sum(
                    (prep["pair_chunk"] >= c0) & (prep["pair_chunk"] < c1)
                )
            ),
        )
    iota_t = np.ascontiguousarray(
        np.tile(
            np.tile(np.arange(P, dtype=np.float32), max_pairs)[None, :], (P, 1)
        ).astype(ml_dtypes.bfloat16)
    )
    npc_pad = nwin * P
    nk = nfeat // P
    w_bf = np.ascontiguousarray(w_sn.astype(ml_dtypes.bfloat16))

    in_maps = []
    for c in range(n_cores):
        xs = x[c * npc : (c + 1) * npc].astype(ml_dtypes.bfloat16)
        xt = np.zeros((P, nk, npc_pad), ml_dtypes.bfloat16)
        xt[:, :, :npc] = np.transpose(xs.reshape(npc, nk, P), (2, 1, 0))
        in_maps.append(
            {
                "x_t": np.ascontiguousarray(xt),
                "w_sn": w_bf,
                "dinv": prep["dinv_cores"][c],
                "bias_t": bias_t,
                "iota_t": iota_t,
                "src_idx": prep["src_cores"][c],
                "tloc": prep["tloc_cores"][c],
            }
        )

    res = run_bass_kernel_spmd(
        nc, in_maps, core_ids=list(range(n_cores)), trace=TRACE
    )
    global LAST_RESULT
    LAST_RESULT = res
    out = np.concatenate(
        [res.results[c]["out_sh"][:npc] for c in range(n_cores)], axis=0
    )
    return out


# revision 34
# speedup vs baseline: 2.0053x; 1.2420x over previous
# GCN encoder (DGI) forward on 8 Trainium2 NeuronCores.
#
# Node-partitioned (graph-parallel) sharding:
#   - nodes are split contiguously across the 8 cores (N/8 per core)
#   - each core owns the edges whose *target* lands in its node range
#   - phase 1: every core computes xw' = dinv[s] * (x_s @ W_sn) for its own
#     nodes (bf16, x pre-transposed on host), keeps it resident in SBUF and
#     AllGathers the full table to DRAM
#   - phase 2: per window, the self-loop lands first via an identity-selector
#     matmul from the resident local table (no DMA); edge messages are
#     gathered with bulk indirect DMA (bf16, 256B/row) from the AllGathered
#     table and scatter-added via one-hot selector matmuls into per-window
#     PSUM; epilogue applies dinv[t]/bias/PReLU.
#   - the gather schedule packs each (window-group, bucket) run contiguously
#     (no per-window chunk padding); chunks straddling window boundaries get
#     one selector matmul per (chunk, window) pair.
#
# Host-side work is limited to input layout (transpose/cast), index
# preprocessing (edge routing/sorting, degree counting) and the tiny
# spectral-norm power iteration on W.

import ml_dtypes
import numpy as np

import concourse.bacc as bacc
import concourse.bass as bass
import concourse.mybir as mybir
import concourse.tile as tile
from concourse.bass_utils import run_bass_kernel_spmd
from concourse.masks import make_identity

P = 128
F32 = mybir.dt.float32
BF16 = mybir.dt.bfloat16
I16 = mybir.dt.int16

# test-harness hooks (ignored in grading)
TRACE = False
LAST_RESULT = None


def _l2n(v, eps=1e-12):
    return v / (np.linalg.norm(v) + eps)


def _spectral_norm_host(W, u):
    W = W.astype(np.float32)
    u = u.astype(np.float32)
    v = _l2n(W.T @ u)
    u2 = _l2n(W @ v)
    sigma = np.float32(u2 @ (W @ v))
    return W / sigma


def _prep_host(x, edge_index, n_cores, win_group, rows_a, max_call_chunks=7):
    """Route edges to cores by target and build the SPMD chunk/pair schedule.

    Slot space: for each window-group g and source bucket b, the edges of
    the group's windows are laid out contiguously: window w owns slots
    [OFF[w], OFF[w]+M[w,b]) where M is the max edge count over cores
    (shared schedule).  Chunks are 128 consecutive slots; a dma_gather call
    covers up to `max_call_chunks` chunks of one (g, b) region.  A matmul
    "pair" is a (chunk, window) with a one-hot selector; chunks that
    straddle window boundaries carry one pair per window.
    """
    n, nfeat = x.shape
    assert n % n_cores == 0
    npc = n // n_cores
    nwin = -(-npc // P)
    # buckets: (table half, core quad) — half A is the first rows_a rows of
    # each core's shard (AllGathered first), half B the rest; four cores'
    # half-blocks are contiguous in the collective output, so one int16
    # gather bucket spans them (4*rows_a < 32768)
    nbuck = 4
    assert 4 * rows_a < 32768 and 4 * (npc - rows_a) < 32768

    row = np.ascontiguousarray(edge_index[0]).astype(np.int64)
    col = np.ascontiguousarray(edge_index[1]).astype(np.int64)

    wkey = (col // npc) * nwin + (col % npc) // P  # global window id
    s_loc = row % npc
    bkt = 2 * (s_loc >= rows_a) + (row // npc) // 4
    key = wkey * nbuck + bkt
    order = np.argsort(key, kind="stable")
    rs = row[order]
    cs = col[order]
    cwb_sorted = key[order]

    deg = 1.0 + np.bincount(col, minlength=n).astype(np.float64)  # + self loop
    dinv_all = (deg ** -0.5).astype(np.float32)

    cnt = np.bincount(key, minlength=n_cores * nwin * nbuck).reshape(
        n_cores, nwin, nbuck
    )
    M = cnt.max(axis=0)  # [nwin, nbuck] shared slot counts

    seg_lo = np.searchsorted(
        cwb_sorted, np.arange(n_cores * nwin * nbuck), side="left"
    )

    # ---- shared schedule ----
    call_sizes = []  # chunks per call
    call_bucket = []
    pair_chunk = []  # global chunk id per pair
    pair_win = []
    pair_lo = []  # slot range of this pair inside its chunk [lo, hi)
    pair_hi = []
    # per-(g,b) bookkeeping for src fill
    regions = []  # (ws, b, OFF dict, L, chunk0)

    # half-A buckets (0, 1) first: their gathers only need the first
    # collective and can start while half B is still exchanging
    border = [0, 1, 2, 3]
    nchunks = 0
    for wg in range(0, nwin, win_group):
        ws = list(range(wg, min(wg + win_group, nwin)))
        for b in border:
            OFF = {}
            L = 0
            for w in ws:
                OFF[w] = L
                L += int(M[w, b])
            if L == 0:
                continue
            nch = -(-L // 128)
            chunk0 = nchunks
            regions.append((ws, b, OFF, L, chunk0))
            q = nch
            while q > 0:
                r = min(q, max_call_chunks)
                call_sizes.append(r)
                call_bucket.append(b)
                q -= r
            for w in ws:
                m = int(M[w, b])
                if m == 0:
                    continue
                k0 = OFF[w] // 128
                k1 = (OFF[w] + m - 1) // 128
                for k in range(k0, k1 + 1):
                    lo = max(OFF[w], k * 128) - k * 128
                    hi = min(OFF[w] + m, (k + 1) * 128) - k * 128
                    pair_chunk.append(chunk0 + k)
                    pair_win.append(w)
                    pair_lo.append(lo)
                    pair_hi.append(hi)
            nchunks += nch

    pair_chunk = np.asarray(pair_chunk)
    pair_win = np.asarray(pair_win)
    npairs = len(pair_chunk)
    assert sum(call_sizes) == nchunks

    # last pair per window (stop + epilogue there); pairs are emitted in
    # schedule order, so a reverse scan suffices
    pair_last = np.zeros(npairs, bool)
    seen = set()
    for i in range(npairs - 1, -1, -1):
        w = int(pair_win[i])
        if w not in seen:
            pair_last[i] = True
            seen.add(w)

    # map call -> first chunk
    call_chunk0 = np.concatenate([[0], np.cumsum(call_sizes)[:-1]])

    # ---- per-core index/selector data ----
    src_cores = []
    tloc_cores = []
    dinv_cores = []
    for c in range(n_cores):
        src_flat = np.zeros(nchunks * P, np.int16)
        tloc_pairs = np.full((npairs, P), -1.0, np.float32)
        for ws, b, OFF, L, chunk0 in regions:
            base = chunk0 * P
            for w in ws:
                m_shared = int(M[w, b])
                if m_shared == 0:
                    continue
                s = c * nwin * nbuck + w * nbuck + b
                i0 = seg_lo[s]
                m = int(cnt[c, w, b])
                if m == 0:
                    continue
                d0 = base + OFF[w]
                h = b // 2
                rows_h = rows_a if h == 0 else npc - rows_a
                src_flat[d0 : d0 + m] = (
                    ((rs[i0 : i0 + m] // npc) % 4) * rows_h
                    + rs[i0 : i0 + m] % npc
                    - h * rows_a
                ).astype(np.int16)
        # fill tloc per pair: iterate regions/windows in schedule order
        pi = 0
        for ws, b, OFF, L, chunk0 in regions:
            for w in ws:
                m_shared = int(M[w, b])
                if m_shared == 0:
                    continue
                s = c * nwin * nbuck + w * nbuck + b
                i0 = seg_lo[s]
                m = int(cnt[c, w, b])
                k0 = OFF[w] // 128
                k1 = (OFF[w] + m_shared - 1) // 128
                for k in range(k0, k1 + 1):
                    lo = max(OFF[w], k * 128) - k * 128
                    hi = min(OFF[w] + m_shared, (k + 1) * 128) - k * 128
                    # slots [lo, hi) of chunk k belong to window w;
                    # core fills first m of the window's m_shared slots
                    gslot0 = k * 128 + lo  # region-relative slot of lo
                    e0 = gslot0 - OFF[w]  # edge offset within window run
                    ne = min(m - e0, hi - lo)
                    assert pair_chunk[pi] == chunk0 + k and pair_win[pi] == w
                    if ne > 0:
                        tloc_pairs[pi, lo : lo + ne] = (
                            cs[i0 + e0 : i0 + e0 + ne] - c * npc - w * P
                        ).astype(np.float32)
                    pi += 1
        assert pi == npairs

        a = src_flat.reshape(nchunks, 8, 16)
        a = np.transpose(a, (2, 0, 1)).reshape(16, nchunks * 8)
        src_cores.append(np.ascontiguousarray(np.tile(a, (8, 1))))
        tloc_cores.append(
            np.ascontiguousarray(tloc_pairs.T.astype(ml_dtypes.bfloat16))
        )

        dv = np.zeros(nwin * P, np.float32)
        dv[:npc] = dinv_all[c * npc : (c + 1) * npc]
        dinv_cores.append(np.ascontiguousarray(dv.reshape(nwin, P).T))

    return dict(
        npc=npc,
        nwin=nwin,
        nbuck=nbuck,
        nchunks=nchunks,
        npairs=npairs,
        call_sizes=call_sizes,
        call_bucket=call_bucket,
        call_chunk0=call_chunk0,
        pair_chunk=pair_chunk,
        pair_win=pair_win,
        pair_last=pair_last,
        win_group=win_group,
        src_cores=src_cores,
        tloc_cores=tloc_cores,
        dinv_cores=dinv_cores,
    )


def _build_nc(
    n,
    nfeat,
    nhid,
    n_cores,
    nwin,
    nbuck,
    rows_a,
    prep,
    alpha,
    bias_is_zero,
    gather_bufs=8,
):
    npc_pad = nwin * P
    npc = n // n_cores
    assert nfeat % P == 0
    nk = nfeat // P

    nchunks = prep["nchunks"]
    npairs = prep["npairs"]
    call_sizes = prep["call_sizes"]
    call_bucket = prep["call_bucket"]
    call_chunk0 = prep["call_chunk0"]
    pair_chunk = prep["pair_chunk"]
    pair_win = prep["pair_win"]
    pair_last = prep["pair_last"]
    win_group = prep["win_group"]

    nc = bacc.Bacc(
        "TRN2",
        target_bir_lowering=False,
        debug=False,
        enable_asserts=False,
        num_devices=n_cores,
        num_swdge_queues=4,
    )

    xt_in = nc.dram_tensor("x_t", [P, nk, npc_pad], BF16, kind="ExternalInput")
    w_in = nc.dram_tensor("w_sn", [nfeat, nhid], BF16, kind="ExternalInput")
    dinv_in = nc.dram_tensor("dinv", [P, nwin], F32, kind="ExternalInput")
    dinva_in = nc.dram_tensor("dinv_a", [P, nwin], F32, kind="ExternalInput")
    bias_in = nc.dram_tensor("bias_t", [P, nhid], F32, kind="ExternalInput")
    max_call = max(call_sizes)
    # pairs per call (for selector batch width)
    pairs_per_call = []
    for ci in range(len(call_sizes)):
        c0 = call_chunk0[ci]
        c1 = c0 + call_sizes[ci]
        pairs_per_call.append(
            int(np.sum((pair_chunk >= c0) & (pair_chunk < c1)))
        )
    max_pairs = max(pairs_per_call)
    iota_in = nc.dram_tensor(
        "iota_t", [P, max_pairs * P], BF16, kind="ExternalInput"
    )
    src_in = nc.dram_tensor("src_idx", [P, nchunks * 8], I16, kind="ExternalInput")
    tloc_in = nc.dram_tensor("tloc", [P, npairs], BF16, kind="ExternalInput")
    out_d = nc.dram_tensor("out_sh", [npc_pad, nhid], F32, kind="ExternalOutput")

    with tile.TileContext(nc) as tc:
        with (
            tc.tile_pool(name="consts", bufs=1) as cpool,
            tc.tile_pool(name="dram", bufs=1, space="DRAM") as dpool,
        ):
            w_sb = cpool.tile([P, nk, nhid], BF16)
            nc.sync.dma_start(
                w_sb[:], w_in[:].rearrange("(k p) h -> p k h", p=P)
            )
            bias_sb = cpool.tile([P, nhid], F32)
            nc.sync.dma_start(bias_sb[:], bias_in[:])
            iota_sb = cpool.tile([P, max_pairs * P], BF16)
            nc.sync.dma_start(iota_sb[:], iota_in[:])
            dinv_sb = cpool.tile([P, nwin], F32)
            nc.sync.dma_start(dinv_sb[:], dinv_in[:])
            dinva_sb = cpool.tile([P, nwin], F32)
            nc.sync.dma_start(dinva_sb[:], dinva_in[:])
            src_sb = cpool.tile([P, nchunks * 8], I16)
            nc.sync.dma_start(src_sb[:], src_in[:])
            tloc_sb = cpool.tile([P, npairs], BF16)
            nc.sync.dma_start(tloc_sb[:], tloc_in[:])
            ident = cpool.tile([P, P], BF16)
            make_identity(nc, ident[:])

            # resident local table (written by phase 1, read by self-loop
            # matmuls) + DRAM staging for the two half-table collectives
            rows_b = npc - rows_a
            wins_a = rows_a // P
            xw_loc = cpool.tile([P, nwin * nhid], BF16)
            ag_in_a = dpool.tile([rows_a, nhid], BF16)
            ag_in_b = dpool.tile([rows_b, nhid], BF16)
            ag_out_a = dpool.tile([n_cores * rows_a, nhid], BF16, addr_space="Shared")
            ag_out_b = dpool.tile([n_cores * rows_b, nhid], BF16, addr_space="Shared")

            # ---- phase 1 ----
            with (
                tc.tile_pool(name="p1xt", bufs=6) as xtpool,
                tc.tile_pool(name="p1pm", bufs=8, space="PSUM") as psumXW,
            ):
                for w in range(nwin):
                    nrow = min(P, npc - w * P)
                    xT = xtpool.tile([P, nk, P], BF16)
                    nc.sync.dma_start(xT[:], xt_in[:, :, w * P : (w + 1) * P])
                    pxw = psumXW.tile([P, nhid], F32)
                    for k in range(nk):
                        nc.tensor.matmul(
                            pxw[:],
                            lhsT=xT[:, k, :],
                            rhs=w_sb[:, k, :],
                            start=(k == 0),
                            stop=(k == nk - 1),
                        )
                    seg = xw_loc[:, w * nhid : (w + 1) * nhid]
                    nc.scalar.activation(
                        out=seg,
                        in_=pxw[:],
                        func=mybir.ActivationFunctionType.Copy,
                        scale=dinv_sb[:, w : w + 1],
                    )
                    # stores ride the ACT engine's HWDGE queue so the Sync
                    # queue only carries the xT loads (phase-1 pacing)
                    if w < wins_a:
                        nc.scalar.dma_start(
                            ag_in_a[w * P : w * P + nrow, :], seg[:nrow, :]
                        )
                    else:
                        r0 = (w - wins_a) * P
                        nc.scalar.dma_start(
                            ag_in_b[r0 : r0 + nrow, :], seg[:nrow, :]
                        )

            nc.gpsimd.collective_compute(
                "AllGather",
                mybir.AluOpType.bypass,
                replica_groups=[list(range(n_cores))],
                ins=[ag_in_a[:]],
                outs=[ag_out_a[:]],
            )
            nc.gpsimd.collective_compute(
                "AllGather",
                mybir.AluOpType.bypass,
                replica_groups=[list(range(n_cores))],
                ins=[ag_in_b[:]],
                outs=[ag_out_b[:]],
            )

            # ---- phase 2 ----
            out_sb = cpool.tile([P, nwin * nhid], F32)
            psum_by_win = {}
            started = set()
            with (
                tc.tile_pool(name="gat", bufs=gather_bufs) as gpool,
                tc.tile_pool(name="sel", bufs=8) as spool,
                tc.tile_pool(name="tmp", bufs=4) as tpool,
                tc.tile_pool(name="acc", bufs=8, space="PSUM") as ppool,
            ):
                pi = 0
                for ci, r in enumerate(call_sizes):
                    c0 = int(call_chunk0[ci])
                    b = int(call_bucket[ci])
                    gbuf = gpool.tile(
                        [P, max_call * nhid], BF16, tag="gbuf", name="gbuf"
                    )
                    h, q = b >> 1, b & 1
                    if h == 0:
                        src_ap = ag_out_a[
                            q * 4 * rows_a : (q + 1) * 4 * rows_a, :
                        ]
                    else:
                        src_ap = ag_out_b[
                            q * 4 * rows_b : (q + 1) * 4 * rows_b, :
                        ]
                    nc.gpsimd.dma_gather(
                        gbuf[:, : r * nhid].rearrange("p (k e) -> p k e", e=nhid),
                        src_ap,
                        src_sb[:, c0 * 8 : (c0 + r) * 8],
                        r * P,
                        r * P,
                        nhid,
                        queue_num=ci % 4,
                    )
                    np_call = pairs_per_call[ci]
                    sel_big = spool.tile(
                        [P, max_pairs * P], BF16, tag="sel", name="sel_big"
                    )
                    nc.vector.tensor_tensor(
                        out=sel_big[:, : np_call * P].rearrange(
                            "p (k e) -> p k e", e=P
                        ),
                        in0=tloc_sb[:, pi : pi + np_call].to_broadcast(
                            [P, np_call, P]
                        ),
                        in1=iota_sb[:, : np_call * P].rearrange(
                            "p (k e) -> p k e", e=P
                        ),
                        op=mybir.AluOpType.is_equal,
                    )
                    for q in range(np_call):
                        i = pi + q
                        w = int(pair_win[i])
                        kk = int(pair_chunk[i]) - c0
                        if w not in started:
                            started.add(w)
                            psum_by_win[w] = ppool.tile(
                                [P, nhid], F32, tag="pw", name="pw"
                            )
                            # self-loop via identity selector from the
                            # resident local table (no DMA involved)
                            nc.tensor.matmul(
                                psum_by_win[w][:],
                                lhsT=ident[:],
                                rhs=xw_loc[:, w * nhid : (w + 1) * nhid],
                                start=True,
                                stop=False,
                            )
                        pw = psum_by_win[w]
                        nc.tensor.matmul(
                            pw[:],
                            lhsT=sel_big[:, q * P : (q + 1) * P],
                            rhs=gbuf[:, kk * nhid : (kk + 1) * nhid],
                            start=False,
                            stop=bool(pair_last[i]),
                        )
                        if pair_last[i]:
                            seg = out_sb[:, w * nhid : (w + 1) * nhid]
                            nc.scalar.activation(
                                out=seg,
                                in_=pw[:],
                                func=mybir.ActivationFunctionType.Copy,
                                scale=dinv_sb[:, w : w + 1],
                            )
                            if not bias_is_zero:
                                nc.vector.tensor_tensor(
                                    out=seg,
                                    in0=seg,
                                    in1=bias_sb[:],
                                    op=mybir.AluOpType.add,
                                )
                            t2 = tpool.tile([P, nhid], F32, tag="t2", name="t2")
                            if 0.0 <= alpha <= 1.0:
                                if bias_is_zero:
                                    # t2 = alpha * dinv_t * pw straight from
                                    # PSUM on ACT (the DVE tensor_scalar path
                                    # measured ~3.7us/op here; ACT ~455ns)
                                    nc.scalar.activation(
                                        out=t2[:],
                                        in_=pw[:],
                                        func=mybir.ActivationFunctionType.Copy,
                                        scale=dinva_sb[:, w : w + 1],
                                    )
                                else:
                                    nc.vector.tensor_scalar_mul(
                                        t2, seg, float(alpha)
                                    )
                                nc.vector.tensor_tensor(
                                    out=seg,
                                    in0=seg,
                                    in1=t2,
                                    op=mybir.AluOpType.max,
                                )
                            else:
                                nc.vector.tensor_scalar(
                                    out=t2,
                                    in0=seg,
                                    scalar1=0.0,
                                    scalar2=float(alpha),
                                    op0=mybir.AluOpType.min,
                                    op1=mybir.AluOpType.mult,
                                )
                                nc.vector.tensor_scalar_max(seg, seg, 0.0)
                                nc.vector.tensor_tensor(
                                    out=seg,
                                    in0=seg,
                                    in1=t2,
                                    op=mybir.AluOpType.add,
                                )
                    pi += np_call
                assert pi == npairs

            nc.sync.dma_start(
                out_d[:].rearrange("(w p) h -> p w h", p=P),
                out_sb[:].rearrange("p (w h) -> p w h", h=nhid),
            )

    nc.compile()
    return nc


def kernel(**inputs):
    x = np.asarray(inputs["x"], dtype=np.float32)
    edge_index = np.asarray(inputs["edge_index"])
    W = np.asarray(inputs["W"], dtype=np.float32)
    bias = np.asarray(inputs["bias"], dtype=np.float32)
    prelu_a = np.asarray(inputs["prelu_a"], dtype=np.float32)
    u = np.asarray(inputs["u"], dtype=np.float32)

    n, nfeat = x.shape
    nhid = W.shape[1]
    n_cores = 8
    win_group = 6
    npc0 = n // n_cores
    # half-A of each core's shard = 49 windows (window-aligned split)
    rows_a = min((-(-npc0 // P) // 2 + 1) * P, npc0)
    alpha = float(prelu_a.reshape(-1)[0])
    bias_is_zero = bool(np.all(bias == 0.0))

    w_sn = _spectral_norm_host(W, u)
    prep = _prep_host(x, edge_index, n_cores, win_group, rows_a, 7)
    npc, nwin = prep["npc"], prep["nwin"]

    nc = _build_nc(
        n,
        nfeat,
        nhid,
        n_cores,
        nwin,
        prep["nbuck"],
        rows_a,
        prep,
        alpha,
        bias_is_zero,
    )

    bias_t = np.ascontiguousarray(np.tile(bias[None, :], (P, 1)))
    # iota sized to the max pairs per call
    max_pairs = 0
    for ci in range(len(prep["call_sizes"])):
        c0 = prep["call_chunk0"][ci]
        c1 = c0 + prep["call_sizes"][ci]
        max_pairs = max(
            max_pairs,
            int(
                np.sum(
                    (prep["pair_chunk"] >= c0) & (prep["pair_chunk"] < c1)
                )
            ),
        )
    iota_t = np.ascontiguousarray(
        np.tile(
            np.tile(np.arange(P, dtype=np.float32), max_pairs)[None, :], (P, 1)
        ).astype(ml_dtypes.bfloat16)
    )
    npc_pad = nwin * P
    nk = nfeat // P
    w_bf = np.ascontiguousarray(w_sn.astype(ml_dtypes.bfloat16))

    in_maps = []
    for c in range(n_cores):
        xs = x[c * npc : (c + 1) * npc].astype(ml_dtypes.bfloat16)
        xt = np.zeros((P, nk, npc_pad), ml_dtypes.bfloat16)
        xt[:, :, :npc] = np.transpose(xs.reshape(npc, nk, P), (2, 1, 0))
        in_maps.append(
            {
                "x_t": np.ascontiguousarray(xt),
                "w_sn": w_bf,
                "dinv": prep["dinv_cores"][c],
                "dinv_a": np.ascontiguousarray(
                    prep["dinv_cores"][c] * np.float32(alpha)
                ),
                "bias_t": bias_t,
                "iota_t": iota_t,
                "src_idx": prep["src_cores"][c],
                "tloc": prep["tloc_cores"][c],
            }
        )

    res = run_bass_kernel_spmd(
        nc, in_maps, core_ids=list(range(n_cores)), trace=TRACE
    )
    global LAST_RESULT
    LAST_RESULT = res
    out = np.concatenate(
        [res.results[c]["out_sh"][:npc] for c in range(n_cores)], axis=0
    )
    return out


# revision 36
# speedup vs baseline: 2.1392x; 1.0668x over previous
# GCN encoder (DGI) forward on 8 Trainium2 NeuronCores.
#
# Node-partitioned (graph-parallel) sharding:
#   - nodes are split contiguously across the 8 cores (N/8 per core)
#   - each core owns the edges whose *target* lands in its node range
#   - phase 1: every core computes xw' = dinv[s] * (x_s @ W_sn) for its own
#     nodes (bf16, x pre-transposed on host), keeps it resident in SBUF and
#     AllGathers the full table to DRAM
#   - phase 2: per window, the self-loop lands first via an identity-selector
#     matmul from the resident local table (no DMA); edge messages are
#     gathered with bulk indirect DMA (bf16, 256B/row) from the AllGathered
#     table and scatter-added via one-hot selector matmuls into per-window
#     PSUM; epilogue applies dinv[t]/bias/PReLU.
#   - the gather schedule packs each (window-group, bucket) run contiguously
#     (no per-window chunk padding); chunks straddling window boundaries get
#     one selector matmul per (chunk, window) pair.
#
# Host-side work is limited to input layout (transpose/cast), index
# preprocessing (edge routing/sorting, degree counting) and the tiny
# spectral-norm power iteration on W.

import ml_dtypes
import numpy as np

import concourse.bacc as bacc
import concourse.bass as bass
import concourse.mybir as mybir
import concourse.tile as tile
from concourse.bass_utils import run_bass_kernel_spmd
from concourse.masks import make_identity

P = 128
F32 = mybir.dt.float32
BF16 = mybir.dt.bfloat16
I16 = mybir.dt.int16

# test-harness hooks (ignored in grading)
TRACE = False
LAST_RESULT = None


def _l2n(v, eps=1e-12):
    return v / (np.linalg.norm(v) + eps)


def _spectral_norm_host(W, u):
    W = W.astype(np.float32)
    u = u.astype(np.float32)
    v = _l2n(W.T @ u)
    u2 = _l2n(W @ v)
    sigma = np.float32(u2 @ (W @ v))
    return W / sigma


def _prep_host(x, edge_index, n_cores, win_group, rows_a, max_call_chunks=7):
    """Route edges to cores by target and build the SPMD chunk/pair schedule.

    Slot space: for each window-group g and source bucket b, the edges of
    the group's windows are laid out contiguously: window w owns slots
    [OFF[w], OFF[w]+M[w,b]) where M is the max edge count over cores
    (shared schedule).  Chunks are 128 consecutive slots; a dma_gather call
    covers up to `max_call_chunks` chunks of one (g, b) region.  A matmul
    "pair" is a (chunk, window) with a one-hot selector; chunks that
    straddle window boundaries carry one pair per window.
    """
    n, nfeat = x.shape
    assert n % n_cores == 0
    npc = n // n_cores
    nwin = -(-npc // P)
    # buckets: (table half, core quad) — half A is the first rows_a rows of
    # each core's shard (AllGathered first), half B the rest; four cores'
    # half-blocks are contiguous in the collective output, so one int16
    # gather bucket spans them (4*rows_a < 32768)
    nbuck = 4
    assert 4 * rows_a < 32768 and 4 * (npc - rows_a) < 32768

    row = np.ascontiguousarray(edge_index[0]).astype(np.int64)
    col = np.ascontiguousarray(edge_index[1]).astype(np.int64)

    wkey = (col // npc) * nwin + (col % npc) // P  # global window id
    s_loc = row % npc
    bkt = 2 * (s_loc >= rows_a) + (row // npc) // 4
    key = wkey * nbuck + bkt
    order = np.argsort(key, kind="stable")
    rs = row[order]
    cs = col[order]
    cwb_sorted = key[order]

    deg = 1.0 + np.bincount(col, minlength=n).astype(np.float64)  # + self loop
    dinv_all = (deg ** -0.5).astype(np.float32)

    cnt = np.bincount(key, minlength=n_cores * nwin * nbuck).reshape(
        n_cores, nwin, nbuck
    )
    M = cnt.max(axis=0)  # [nwin, nbuck] shared slot counts

    seg_lo = np.searchsorted(
        cwb_sorted, np.arange(n_cores * nwin * nbuck), side="left"
    )

    # ---- shared schedule ----
    call_sizes = []  # chunks per call
    call_bucket = []
    pair_chunk = []  # global chunk id per pair
    pair_win = []
    pair_lo = []  # slot range of this pair inside its chunk [lo, hi)
    pair_hi = []
    # per-(g,b) bookkeeping for src fill
    regions = []  # (ws, b, OFF dict, L, chunk0)

    # half-A buckets (0, 1) first: their gathers only need the first
    # collective and can start while half B is still exchanging
    border = [0, 1, 2, 3]
    nchunks = 0
    for wg in range(0, nwin, win_group):
        ws = list(range(wg, min(wg + win_group, nwin)))
        for b in border:
            OFF = {}
            L = 0
            for w in ws:
                OFF[w] = L
                L += int(M[w, b])
            if L == 0:
                continue
            nch = -(-L // 128)
            chunk0 = nchunks
            regions.append((ws, b, OFF, L, chunk0))
            q = nch
            while q > 0:
                r = min(q, max_call_chunks)
                call_sizes.append(r)
                call_bucket.append(b)
                q -= r
            for w in ws:
                m = int(M[w, b])
                if m == 0:
                    continue
                k0 = OFF[w] // 128
                k1 = (OFF[w] + m - 1) // 128
                for k in range(k0, k1 + 1):
                    lo = max(OFF[w], k * 128) - k * 128
                    hi = min(OFF[w] + m, (k + 1) * 128) - k * 128
                    pair_chunk.append(chunk0 + k)
                    pair_win.append(w)
                    pair_lo.append(lo)
                    pair_hi.append(hi)
            nchunks += nch

    pair_chunk = np.asarray(pair_chunk)
    pair_win = np.asarray(pair_win)
    npairs = len(pair_chunk)
    assert sum(call_sizes) == nchunks

    # last pair per window (stop + epilogue there); pairs are emitted in
    # schedule order, so a reverse scan suffices
    pair_last = np.zeros(npairs, bool)
    seen = set()
    for i in range(npairs - 1, -1, -1):
        w = int(pair_win[i])
        if w not in seen:
            pair_last[i] = True
            seen.add(w)

    # map call -> first chunk
    call_chunk0 = np.concatenate([[0], np.cumsum(call_sizes)[:-1]])

    # ---- per-core index/selector data ----
    src_cores = []
    tloc_cores = []
    dinv_cores = []
    for c in range(n_cores):
        src_flat = np.zeros(nchunks * P, np.int16)
        tloc_pairs = np.full((npairs, P), -1.0, np.float32)
        for ws, b, OFF, L, chunk0 in regions:
            base = chunk0 * P
            for w in ws:
                m_shared = int(M[w, b])
                if m_shared == 0:
                    continue
                s = c * nwin * nbuck + w * nbuck + b
                i0 = seg_lo[s]
                m = int(cnt[c, w, b])
                if m == 0:
                    continue
                d0 = base + OFF[w]
                h = b // 2
                rows_h = rows_a if h == 0 else npc - rows_a
                src_flat[d0 : d0 + m] = (
                    ((rs[i0 : i0 + m] // npc) % 4) * rows_h
                    + rs[i0 : i0 + m] % npc
                    - h * rows_a
                ).astype(np.int16)
        # fill tloc per pair: iterate regions/windows in schedule order
        pi = 0
        for ws, b, OFF, L, chunk0 in regions:
            for w in ws:
                m_shared = int(M[w, b])
                if m_shared == 0:
                    continue
                s = c * nwin * nbuck + w * nbuck + b
                i0 = seg_lo[s]
                m = int(cnt[c, w, b])
                k0 = OFF[w] // 128
                k1 = (OFF[w] + m_shared - 1) // 128
                for k in range(k0, k1 + 1):
                    lo = max(OFF[w], k * 128) - k * 128
                    hi = min(OFF[w] + m_shared, (k + 1) * 128) - k * 128
                    # slots [lo, hi) of chunk k belong to window w;
                    # core fills first m of the window's m_shared slots
                    gslot0 = k * 128 + lo  # region-relative slot of lo
                    e0 = gslot0 - OFF[w]  # edge offset within window run
                    ne = min(m - e0, hi - lo)
                    assert pair_chunk[pi] == chunk0 + k and pair_win[pi] == w
                    if ne > 0:
                        tloc_pairs[pi, lo : lo + ne] = (
                            cs[i0 + e0 : i0 + e0 + ne] - c * npc - w * P
                        ).astype(np.float32)
                    pi += 1
        assert pi == npairs

        a = src_flat.reshape(nchunks, 8, 16)
        a = np.transpose(a, (2, 0, 1)).reshape(16, nchunks * 8)
        src_cores.append(np.ascontiguousarray(np.tile(a, (8, 1))))
        tloc_cores.append(
            np.ascontiguousarray(tloc_pairs.T.astype(ml_dtypes.bfloat16))
        )

        dv = np.zeros(nwin * P, np.float32)
        dv[:npc] = dinv_all[c * npc : (c + 1) * npc]
        dinv_cores.append(np.ascontiguousarray(dv.reshape(nwin, P).T))

    return dict(
        npc=npc,
        nwin=nwin,
        nbuck=nbuck,
        nchunks=nchunks,
        npairs=npairs,
        call_sizes=call_sizes,
        call_bucket=call_bucket,
        call_chunk0=call_chunk0,
        pair_chunk=pair_chunk,
        pair_win=pair_win,
        pair_last=pair_last,
        win_group=win_group,
        src_cores=src_cores,
        tloc_cores=tloc_cores,
        dinv_cores=dinv_cores,
    )


def _build_nc(
    n,
    nfeat,
    nhid,
    n_cores,
    nwin,
    nbuck,
    rows_a,
    prep,
    alpha,
    bias_is_zero,
    gather_bufs=8,
):
    npc_pad = nwin * P
    npc = n // n_cores
    assert nfeat % P == 0
    nk = nfeat // P

    nchunks = prep["nchunks"]
    npairs = prep["npairs"]
    call_sizes = prep["call_sizes"]
    call_bucket = prep["call_bucket"]
    call_chunk0 = prep["call_chunk0"]
    pair_chunk = prep["pair_chunk"]
    pair_win = prep["pair_win"]
    pair_last = prep["pair_last"]
    win_group = prep["win_group"]

    nc = bacc.Bacc(
        "TRN2",
        target_bir_lowering=False,
        debug=False,
        enable_asserts=False,
        num_devices=n_cores,
        num_swdge_queues=4,
    )

    xt_in = nc.dram_tensor("x_t", [P, nk, npc_pad], BF16, kind="ExternalInput")
    w_in = nc.dram_tensor("w_sn", [nfeat, nhid], BF16, kind="ExternalInput")
    dinv_in = nc.dram_tensor("dinv", [P, nwin], F32, kind="ExternalInput")
    dinva_in = nc.dram_tensor("dinv_a", [P, nwin], F32, kind="ExternalInput")
    bias_in = nc.dram_tensor("bias_t", [P, nhid], F32, kind="ExternalInput")
    max_call = max(call_sizes)
    # pairs per call (for selector batch width)
    pairs_per_call = []
    for ci in range(len(call_sizes)):
        c0 = call_chunk0[ci]
        c1 = c0 + call_sizes[ci]
        pairs_per_call.append(
            int(np.sum((pair_chunk >= c0) & (pair_chunk < c1)))
        )
    max_pairs = max(pairs_per_call)
    iota_in = nc.dram_tensor(
        "iota_t", [P, max_pairs * P], BF16, kind="ExternalInput"
    )
    src_in = nc.dram_tensor("src_idx", [P, nchunks * 8], I16, kind="ExternalInput")
    tloc_in = nc.dram_tensor("tloc", [P, npairs], BF16, kind="ExternalInput")
    out_d = nc.dram_tensor("out_sh", [npc_pad, nhid], F32, kind="ExternalOutput")

    with tile.TileContext(nc) as tc:
        with (
            tc.tile_pool(name="consts", bufs=1) as cpool,
            tc.tile_pool(name="dram", bufs=1, space="DRAM") as dpool,
        ):
            w_sb = cpool.tile([P, nk, nhid], BF16)
            nc.sync.dma_start(
                w_sb[:], w_in[:].rearrange("(k p) h -> p k h", p=P)
            )
            bias_sb = cpool.tile([P, nhid], F32)
            nc.sync.dma_start(bias_sb[:], bias_in[:])
            iota_sb = cpool.tile([P, max_pairs * P], BF16)
            nc.sync.dma_start(iota_sb[:], iota_in[:])
            dinv_sb = cpool.tile([P, nwin], F32)
            nc.sync.dma_start(dinv_sb[:], dinv_in[:])
            dinva_sb = cpool.tile([P, nwin], F32)
            nc.sync.dma_start(dinva_sb[:], dinva_in[:])
            src_sb = cpool.tile([P, nchunks * 8], I16)
            nc.sync.dma_start(src_sb[:], src_in[:])
            tloc_sb = cpool.tile([P, npairs], BF16)
            nc.sync.dma_start(tloc_sb[:], tloc_in[:])
            ident = cpool.tile([P, P], BF16)
            make_identity(nc, ident[:])

            # resident local table (written by phase 1, read by self-loop
            # matmuls) + DRAM staging for the two half-table collectives
            rows_b = npc - rows_a
            wins_a = rows_a // P
            xw_loc = cpool.tile([P, nwin * nhid], BF16)
            ag_in_a = dpool.tile([rows_a, nhid], BF16)
            ag_in_b = dpool.tile([rows_b, nhid], BF16)
            ag_out_a = dpool.tile([n_cores * rows_a, nhid], BF16, addr_space="Shared")
            ag_out_b = dpool.tile([n_cores * rows_b, nhid], BF16, addr_space="Shared")

            # ---- phase 1 ----
            # window-pair batching halves the load/store count; wins_a is
            # even so pairs never straddle the A/B table split
            assert wins_a % 2 == 0 and nwin % 2 == 0
            with (
                tc.tile_pool(name="p1xt", bufs=6) as xtpool,
                tc.tile_pool(name="p1pm", bufs=8, space="PSUM") as psumXW,
            ):
                for w in range(0, nwin, 2):
                    nrow2 = min(2 * P, npc - w * P)
                    xT = xtpool.tile([P, nk, 2 * P], BF16)
                    nc.sync.dma_start(xT[:], xt_in[:, :, w * P : (w + 2) * P])
                    pxw = psumXW.tile([P, 2 * nhid], F32)
                    for half in range(2):
                        for k in range(nk):
                            nc.tensor.matmul(
                                pxw[:, half * nhid : (half + 1) * nhid],
                                lhsT=xT[:, k, half * P : (half + 1) * P],
                                rhs=w_sb[:, k, :],
                                start=(k == 0),
                                stop=(k == nk - 1),
                            )
                        nc.scalar.activation(
                            out=xw_loc[
                                :, (w + half) * nhid : (w + half + 1) * nhid
                            ],
                            in_=pxw[:, half * nhid : (half + 1) * nhid],
                            func=mybir.ActivationFunctionType.Copy,
                            scale=dinv_sb[:, w + half : w + half + 1],
                        )
                    seg2 = xw_loc[:, w * nhid : (w + 2) * nhid].rearrange(
                        "p (v h) -> p v h", h=nhid
                    )
                    # stores ride the ACT engine's HWDGE queue so the Sync
                    # queue only carries the xT loads (phase-1 pacing)
                    if w < wins_a:
                        nc.scalar.dma_start(
                            ag_in_a[w * P : w * P + nrow2, :].rearrange(
                                "(v p) h -> p v h", p=P
                            )
                            if nrow2 == 2 * P
                            else ag_in_a[w * P : w * P + nrow2, :],
                            seg2
                            if nrow2 == 2 * P
                            else xw_loc[:, w * nhid : (w + 1) * nhid][
                                :nrow2, :
                            ],
                        )
                    else:
                        r0 = (w - wins_a) * P
                        if nrow2 == 2 * P:
                            nc.scalar.dma_start(
                                ag_in_b[r0 : r0 + nrow2, :].rearrange(
                                    "(v p) h -> p v h", p=P
                                ),
                                seg2,
                            )
                        else:
                            # tail: first full window then the partial one
                            nr1 = min(P, nrow2)
                            nc.scalar.dma_start(
                                ag_in_b[r0 : r0 + nr1, :],
                                xw_loc[:, w * nhid : (w + 1) * nhid][:nr1, :],
                            )
                            if nrow2 > P:
                                nc.scalar.dma_start(
                                    ag_in_b[r0 + P : r0 + nrow2, :],
                                    xw_loc[
                                        :, (w + 1) * nhid : (w + 2) * nhid
                                    ][: nrow2 - P, :],
                                )

            nc.gpsimd.collective_compute(
                "AllGather",
                mybir.AluOpType.bypass,
                replica_groups=[list(range(n_cores))],
                ins=[ag_in_a[:]],
                outs=[ag_out_a[:]],
            )
            nc.gpsimd.collective_compute(
                "AllGather",
                mybir.AluOpType.bypass,
                replica_groups=[list(range(n_cores))],
                ins=[ag_in_b[:]],
                outs=[ag_out_b[:]],
            )

            # ---- phase 2 ----
            out_sb = cpool.tile([P, nwin * nhid], F32)
            psum_by_win = {}
            started = set()
            with (
                tc.tile_pool(name="gat", bufs=gather_bufs) as gpool,
                tc.tile_pool(name="sel", bufs=8) as spool,
                tc.tile_pool(name="tmp", bufs=4) as tpool,
                tc.tile_pool(name="acc", bufs=8, space="PSUM") as ppool,
            ):
                pi = 0
                for ci, r in enumerate(call_sizes):
                    c0 = int(call_chunk0[ci])
                    b = int(call_bucket[ci])
                    gbuf = gpool.tile(
                        [P, max_call * nhid], BF16, tag="gbuf", name="gbuf"
                    )
                    h, q = b >> 1, b & 1
                    if h == 0:
                        src_ap = ag_out_a[
                            q * 4 * rows_a : (q + 1) * 4 * rows_a, :
                        ]
                    else:
                        src_ap = ag_out_b[
                            q * 4 * rows_b : (q + 1) * 4 * rows_b, :
                        ]
                    nc.gpsimd.dma_gather(
                        gbuf[:, : r * nhid].rearrange("p (k e) -> p k e", e=nhid),
                        src_ap,
                        src_sb[:, c0 * 8 : (c0 + r) * 8],
                        r * P,
                        r * P,
                        nhid,
                        queue_num=ci % 4,
                    )
                    np_call = pairs_per_call[ci]
                    sel_big = spool.tile(
                        [P, max_pairs * P], BF16, tag="sel", name="sel_big"
                    )
                    nc.vector.tensor_tensor(
                        out=sel_big[:, : np_call * P].rearrange(
                            "p (k e) -> p k e", e=P
                        ),
                        in0=tloc_sb[:, pi : pi + np_call].to_broadcast(
                            [P, np_call, P]
                        ),
                        in1=iota_sb[:, : np_call * P].rearrange(
                            "p (k e) -> p k e", e=P
                        ),
                        op=mybir.AluOpType.is_equal,
                    )
                    for q in range(np_call):
                        i = pi + q
                        w = int(pair_win[i])
                        kk = int(pair_chunk[i]) - c0
                        if w not in started:
                            started.add(w)
                            psum_by_win[w] = ppool.tile(
                                [P, nhid], F32, tag="pw", name="pw"
                            )
                            # self-loop via identity selector from the
                            # resident local table (no DMA involved)
                            nc.tensor.matmul(
                                psum_by_win[w][:],
                                lhsT=ident[:],
                                rhs=xw_loc[:, w * nhid : (w + 1) * nhid],
                                start=True,
                                stop=False,
                            )
                        pw = psum_by_win[w]
                        nc.tensor.matmul(
                            pw[:],
                            lhsT=sel_big[:, q * P : (q + 1) * P],
                            rhs=gbuf[:, kk * nhid : (kk + 1) * nhid],
                            start=False,
                            stop=bool(pair_last[i]),
                        )
                        if pair_last[i]:
                            seg = out_sb[:, w * nhid : (w + 1) * nhid]
                            nc.scalar.activation(
                                out=seg,
                                in_=pw[:],
                                func=mybir.ActivationFunctionType.Copy,
                                scale=dinv_sb[:, w : w + 1],
                            )
                            if not bias_is_zero:
                                nc.vector.tensor_tensor(
                                    out=seg,
                                    in0=seg,
                                    in1=bias_sb[:],
                                    op=mybir.AluOpType.add,
                                )
                            t2 = tpool.tile([P, nhid], F32, tag="t2", name="t2")
                            if 0.0 <= alpha <= 1.0:
                                if bias_is_zero:
                                    # t2 = alpha * dinv_t * pw straight from
                                    # PSUM on ACT (the DVE tensor_scalar path
                                    # measured ~3.7us/op here; ACT ~455ns)
                                    nc.scalar.activation(
                                        out=t2[:],
                                        in_=pw[:],
                                        func=mybir.ActivationFunctionType.Copy,
                                        scale=dinva_sb[:, w : w + 1],
                                    )
                                else:
                                    nc.vector.tensor_scalar_mul(
                                        t2, seg, float(alpha)
                                    )
                                nc.vector.tensor_tensor(
                                    out=seg,
                                    in0=seg,
                                    in1=t2,
                                    op=mybir.AluOpType.max,
                                )
                            else:
                                nc.vector.tensor_scalar(
                                    out=t2,
                                    in0=seg,
                                    scalar1=0.0,
                                    scalar2=float(alpha),
                                    op0=mybir.AluOpType.min,
                                    op1=mybir.AluOpType.mult,
                                )
                                nc.vector.tensor_scalar_max(seg, seg, 0.0)
                                nc.vector.tensor_tensor(
                                    out=seg,
                                    in0=seg,
                                    in1=t2,
                                    op=mybir.AluOpType.add,
                                )
                            # stream the finished window out on the (idle)
                            # Sync queue instead of one big tail store
                            nc.sync.dma_start(
                                out_d[w * P : (w + 1) * P, :], seg
                            )
                    pi += np_call
                assert pi == npairs

    nc.compile()
    return nc


def kernel(**inputs):
    x = np.asarray(inputs["x"], dtype=np.float32)
    edge_index = np.asarray(inputs["edge_index"])
    W = np.asarray(inputs["W"], dtype=np.float32)
    bias = np.asarray(inputs["bias"], dtype=np.float32)
    prelu_a = np.asarray(inputs["prelu_a"], dtype=np.float32)
    u = np.asarray(inputs["u"], dtype=np.float32)

    n, nfeat = x.shape
    nhid = W.shape[1]
    n_cores = 8
    win_group = 6
    npc0 = n // n_cores
    # half-A of each core's shard = 49 windows (window-aligned split)
    rows_a = min((-(-npc0 // P) // 2 + 1) * P, npc0)
    alpha = float(prelu_a.reshape(-1)[0])
    bias_is_zero = bool(np.all(bias == 0.0))

    w_sn = _spectral_norm_host(W, u)
    prep = _prep_host(x, edge_index, n_cores, win_group, rows_a, 7)
    npc, nwin = prep["npc"], prep["nwin"]

    nc = _build_nc(
        n,
        nfeat,
        nhid,
        n_cores,
        nwin,
        prep["nbuck"],
        rows_a,
        prep,
        alpha,
        bias_is_zero,
    )

    bias_t = np.ascontiguousarray(np.tile(bias[None, :], (P, 1)))
    # iota sized to the max pairs per call
    max_pairs = 0
    for ci in range(len(prep["call_sizes"])):
        c0 = prep["call_chunk0"][ci]
        c1 = c0 + prep["call_sizes"][ci]
        max_pairs = max(
            max_pairs,
            int(
                np.sum(
                    (prep["pair_chunk"] >= c0) & (prep["pair_chunk"] < c1)
                )
            ),
        )
    iota_t = np.ascontiguousarray(
        np.tile(
            np.tile(np.arange(P, dtype=np.float32), max_pairs)[None, :], (P, 1)
        ).astype(ml_dtypes.bfloat16)
    )
    npc_pad = nwin * P
    nk = nfeat // P
    w_bf = np.ascontiguousarray(w_sn.astype(ml_dtypes.bfloat16))

    in_maps = []
    for c in range(n_cores):
        xs = x[c * npc : (c + 1) * npc].astype(ml_dtypes.bfloat16)
        xt = np.zeros((P, nk, npc_pad), ml_dtypes.bfloat16)
        xt[:, :, :npc] = np.transpose(xs.reshape(npc, nk, P), (2, 1, 0))
        in_maps.append(
            {
                "x_t": np.ascontiguousarray(xt),
                "w_sn": w_bf,
                "dinv": prep["dinv_cores"][c],
                "dinv_a": np.ascontiguousarray(
                    prep["dinv_cores"][c] * np.float32(alpha)
                ),
                "bias_t": bias_t,
                "iota_t": iota_t,
                "src_idx": prep["src_cores"][c],
                "tloc": prep["tloc_cores"][c],
            }
        )

    res = run_bass_kernel_spmd(
        nc, in_maps, core_ids=list(range(n_cores)), trace=TRACE
    )
    global LAST_RESULT
    LAST_RESULT = res
    out = np.concatenate(
        [res.results[c]["out_sh"][:npc] for c in range(n_cores)], axis=0
    )
    return out
